# revision 20
# baseline (speedup 1.0000x reference)
"""Trainium2 Bass kernel for nn_EquivariantMessagePasser (8-core SPMD).

Strategy: edges sorted+sharded by center atom (segment-sum is core-local via
per-tile indicator matmuls into accumulating PSUM); feats replicated; per-atom
uncoupled-feature table built on device in DRAM and gathered per edge via
indirect DMA; couple-back + output linear folded into one host-precomputed
weight (WU = U x Wl).
"""
import sys

sys.path.insert(0, "/opt/trn_rl_repo")

from contextlib import ExitStack

import numpy as np
import ml_dtypes

import concourse.bass as bass
from concourse import bacc, mybir
from concourse.bass import IndirectOffsetOnAxis
from concourse.tile import TileContext
from concourse.bass_utils import run_bass_kernel_spmd
from concourse.masks import make_identity

F32 = mybir.dt.float32
BF16 = mybir.dt.bfloat16
I32 = mybir.dt.int32
bf = ml_dtypes.bfloat16

NMAX = [8, 6, 4, 2]
KMAX = [128, 96, 64, 32]
PADDED_L = [0, 2, 2, 4]
SIDE = [1, 3, 3, 5]
MSZ = [1, 4, 9, 16]
ITSZ = [1, 9, 9, 25]
N_ATOMS = 2500
HIDDEN = 64
NC_ = 8
KW = 32
LO = [96, 64, 32, 0]
KOFF = [0, 128, 224, 288]         # radial region offsets (cumsum KMAX)

# row layout: [l3 (i/t,j,k)=800 | l2 288 | l1 288 | l0 32] = 1408
GOFF = {3: 0, 2: 800, 1: 1088, 0: 1376}
GROW = 1408
PTOT = 1408
NPIECE = 11

VKB = {3: 8, 2: 8, 1: 32}         # k-channels per uncouple-V matmul
TBB = {3: 5, 2: 14, 1: 14}        # k-channels per table-build block


def _uflat(U):
    side = U.shape[0]
    return U.reshape(side * side, side * side).T.copy()  # [m, (i,j)]


def _blockdiag(mat, B):
    m, n = mat.shape
    out = np.zeros((B * m, B * n), mat.dtype)
    for b in range(B):
        out[b * m:(b + 1) * m, b * n:(b + 1) * n] = mat
    return out


def _cfeat(feats, l):
    return np.concatenate(
        [feats[lp][:, :, LO[l]:LO[l] + KW] for lp in range(l + 1)], axis=1)


def _build_wu(U, Wl):
    u0 = float(np.asarray(U[0]).reshape(-1)[0])
    wu = []
    for l in range(4):
        ncol = (2 * l + 1) * KMAX[l]
        M = np.zeros((PTOT, ncol), np.float32)
        for lch in range(l, 4):
            s = SIDE[lch]
            uf = np.asarray(U[PADDED_L[lch]], np.float32)
            koff = (lch - l) * KW
            for i in range(s):
                for j in range(s):
                    for mloc in range(2 * l + 1):
                        uv = float(uf[i, j, l * l + mloc])
                        if lch == 0:
                            uv *= u0 * u0
                        r0 = GOFF[lch] + (i * s + j) * KW
                        M[r0:r0 + KW, mloc * KMAX[l]:(mloc + 1) * KMAX[l]] += \
                            uv * Wl[l][koff:koff + KW, :]
        wu.append(M)
    return wu


def _host_prep(inp):
    rb = [np.asarray(inp[f'radial_basis_{l}'], np.float32) for l in range(4)]
    sph = [np.asarray(inp[f'spherical_harmonics_{l}'], np.float32)
           for l in range(4)]
    feats = [np.asarray(inp[f'features_{l}'], np.float32) for l in range(4)]
    centers = np.asarray(inp['centers'])
    neighbors = np.asarray(inp['neighbors'])
    U = {L: np.asarray(inp[f'U{L}'], np.float32) for L in (0, 2, 4)}
    Wr1 = [np.asarray(inp[f'Wr1_{l}'], np.float32) for l in range(4)]
    Wr2 = [np.asarray(inp[f'Wr2_{l}'], np.float32) for l in range(4)]
    Wl = [np.asarray(inp[f'Wl_{l}'], np.float32) for l in range(4)]

    order = np.argsort(centers, kind='stable')
    c_sorted = centers[order]
    abnd = [round(c * N_ATOMS / NC_) for c in range(NC_ + 1)]
    starts = np.searchsorted(c_sorted, np.arange(N_ATOMS + 1))
    core_chunks = []
    nch_max = 0
    for c in range(NC_):
        a0c, a1c = abnd[c], abnd[c + 1]
        chunks = []
        a = a0c
        while a < a1c:
            na = min(128, a1c - a)
            e0, e1 = int(starts[a]), int(starts[a + na])
            chunks.append((a, na, e0, e1 - e0))
            a += na
        core_chunks.append(chunks)
        nch_max = max(nch_max, len(chunks))
    NCH = nch_max
    for c in range(NC_):
        while len(core_chunks[c]) < NCH:
            core_chunks[c].append((abnd[c + 1], 0, 0, 0))
    nt_s = [max((core_chunks[c][s][3] + 127) // 128 for c in range(NC_))
            for s in range(NCH)]
    NT = sum(nt_s)
    EP = NT * 128
    tbase = np.cumsum([0] + nt_s)

    sph_cat = np.concatenate(sph, axis=1)

    per_core = []
    for c in range(NC_):
        eidx = np.zeros(EP, np.int64)
        valid = np.zeros(EP, np.float32)
        ind = np.zeros((NT, 128, 128), np.float32)
        for s_i, (a0, na, e0, ne) in enumerate(core_chunks[c]):
            pos0 = int(tbase[s_i]) * 128
            idx = order[e0:e0 + ne]
            eidx[pos0:pos0 + ne] = idx
            valid[pos0:pos0 + ne] = 1.0
            loc = c_sorted[e0:e0 + ne] - a0
            rows = np.arange(pos0, pos0 + ne)
            ind[rows // 128, rows % 128, loc] = 1.0
        d = {}
        for l in range(4):
            d[f'rbT_{l}'] = np.ascontiguousarray(
                (rb[l][eidx] * valid[:, None]).T).astype(bf)
        d['s_mat'] = (sph_cat[eidx] * valid[:, None]).astype(np.float32)
        d['nbr'] = np.ascontiguousarray(
            (neighbors[eidx] * valid.astype(np.int64)).astype(np.int32)
            [:, None])
        d['ind'] = ind.reshape(NT * 128, 128).astype(bf)
        a0c, a1c = abnd[c], abnd[c + 1]
        for l in range(4):
            fo = np.zeros((NCH * 128, (2 * l + 1) * KMAX[l]), np.float32)
            fo[:a1c - a0c] = feats[l][a0c:a1c].reshape(a1c - a0c, -1)
            d[f'featown_{l}'] = fo
        per_core.append(d)

    rep = {}
    for l in (1, 2, 3):
        B, m = TBB[l], MSZ[l]
        cf = _cfeat(feats, l)
        ng = (KW + B - 1) // B
        t = np.zeros((B * m, ng, N_ATOMS), np.float32)
        for g in range(ng):
            for kb in range(B):
                k = g * B + kb
                if k < KW:
                    t[kb * m:(kb + 1) * m, g, :] = cf[:, :, k].T
        rep[f'cfT_{l}'] = t.astype(bf)
        uf = _uflat(U[PADDED_L[l]])[:m, :]
        rep[f'Utab_{l}'] = _blockdiag(uf, B).astype(bf)
        rep[f'UVbd_{l}'] = _blockdiag(uf, VKB[l]).astype(bf)
    rep['g0tab'] = np.ascontiguousarray(feats[0][:, 0, 96:128]).astype(bf)
    wu = _build_wu(U, Wl)
    for l in range(4):
        rep[f'WU_{l}'] = wu[l].astype(bf)
        rep[f'Wr1_{l}'] = Wr1[l].astype(bf)
        rep[f'Wr2_{l}'] = Wr2[l].astype(bf)

    meta = dict(NT=NT, NCH=NCH, nt_s=nt_s, tbase=[int(x) for x in tbase],
                abnd=abnd, EP=EP)
    return per_core, rep, meta


def build_program(meta):
    import os
    PH = int(os.environ.get("KPHASE", "4"))
    NT, NCH, EP = meta['NT'], meta['NCH'], meta['EP']
    nt_s, tbase = meta['nt_s'], meta['tbase']

    nc = bacc.Bacc("TRN2", target_bir_lowering=False, debug=False,
                   num_devices=NC_)
    ctx = ExitStack()

    din = {}

    def dri(name, shape, dt):
        din[name] = nc.dram_tensor(name, shape, dt, kind="ExternalInput")

    for l in range(4):
        dri(f'rbT_{l}', [NMAX[l], EP], BF16)
        dri(f'featown_{l}', [NCH * 128, (2 * l + 1) * KMAX[l]], F32)
        dri(f'WU_{l}', [PTOT, (2 * l + 1) * KMAX[l]], BF16)
        dri(f'Wr1_{l}', [NMAX[l], HIDDEN], BF16)
        dri(f'Wr2_{l}', [HIDDEN, KMAX[l]], BF16)
    dri('s_mat', [EP, 16], F32)
    dri('nbr', [EP, 1], I32)
    dri('ind', [NT * 128, 128], BF16)
    for l in (1, 2, 3):
        B, m = TBB[l], MSZ[l]
        ng = (KW + B - 1) // B
        dri(f'cfT_{l}', [B * m, ng, N_ATOMS], BF16)
        dri(f'Utab_{l}', [B * m, B * SIDE[l] ** 2], BF16)
        dri(f'UVbd_{l}', [VKB[l] * m, VKB[l] * ITSZ[l]], BF16)
    dri('g0tab', [N_ATOMS, KW], BF16)
    douts = [nc.dram_tensor(f'out_{l}', [NCH * 128, (2 * l + 1) * KMAX[l]],
                            F32, kind="ExternalOutput") for l in range(4)]
    gtab = nc.dram_tensor('gtab', [N_ATOMS, GROW], BF16)

    with TileContext(nc) as tc:
        cpool = ctx.enter_context(tc.tile_pool(name="const", bufs=1))
        ident = cpool.tile([128, 128], BF16)
        make_identity(nc, ident[:])

        sb = {}
        for name in ('Utab_1', 'Utab_2', 'Utab_3', 'UVbd_1', 'UVbd_2',
                     'UVbd_3', 'Wr1_0', 'Wr1_1', 'Wr1_2', 'Wr1_3',
                     'Wr2_0', 'Wr2_1', 'Wr2_2', 'Wr2_3'):
            t = din[name]
            sb[name] = cpool.tile(list(t.shape), t.dtype, name=name)
            nc.sync.dma_start(sb[name][:], t.ap())
        for l in range(4):
            t = din[f'WU_{l}']
            w = cpool.tile([128, NPIECE, t.shape[1]], BF16, name=f'wu{l}')
            sb[f'WU_{l}'] = w
            nc.sync.dma_start(w[:],
                              t.ap().rearrange("(p q) c -> q p c", q=128))
        s_sb = cpool.tile([128, NT, 16], F32)
        nc.sync.dma_start(
            s_sb[:], din['s_mat'].ap().rearrange("(t q) m -> q t m", q=128))
        nbr_all = cpool.tile([128, NT], I32)
        nc.sync.dma_start(
            nbr_all[:], din['nbr'].ap().rearrange("(t q) one -> q (t one)",
                                                  q=128))
        ind_all = cpool.tile([128, NT, 128], BF16)
        nc.sync.dma_start(
            ind_all[:], din['ind'].ap().rearrange("(t q) a -> q t a", q=128))
        rbT_sb = {}
        for l in range(4):
            rbT_sb[l] = cpool.tile([NMAX[l], EP], BF16, name=f'rbt{l}')
            nc.sync.dma_start(rbT_sb[l][:], din[f'rbT_{l}'].ap())

        # -------- phase 1: radial MLP --------
        radial_sb = cpool.tile([128, NT, 320], F32)
        if PH >= 1:
         with tc.tile_pool(name="p1ps", bufs=2, space="PSUM") as pp1, \
                tc.tile_pool(name="p1sb", bufs=2) as ps1:
            for l in range(4):
                for t0 in range(0, NT, 4):
                    nt4 = min(4, NT - t0)
                    ec = nt4 * 128
                    h_ps = pp1.tile([HIDDEN, 512], F32, tag="h")
                    nc.tensor.matmul(h_ps[:, :ec], lhsT=sb[f'Wr1_{l}'][:],
                                     rhs=rbT_sb[l][:, t0 * 128:t0 * 128 + ec],
                                     start=True, stop=True)
                    h_sg = ps1.tile([HIDDEN, 512], F32, tag="hsg")
                    nc.scalar.activation(h_sg[:, :ec], h_ps[:, :ec],
                                         mybir.ActivationFunctionType.Sigmoid)
                    h_sb = ps1.tile([HIDDEN, 512], BF16, tag="hs")
                    nc.vector.tensor_tensor(out=h_sb[:, :ec],
                                            in0=h_ps[:, :ec],
                                            in1=h_sg[:, :ec],
                                            op=mybir.AluOpType.mult)
                    for ti in range(nt4):
                        r_ps = pp1.tile([128, KMAX[l]], F32, tag="r")
                        nc.tensor.matmul(r_ps[:],
                                         lhsT=h_sb[:, ti * 128:ti * 128 + 128],
                                         rhs=sb[f'Wr2_{l}'][:],
                                         start=True, stop=True)
                        nc.scalar.copy(
                            radial_sb[:, t0 + ti,
                                      KOFF[l]:KOFF[l] + KMAX[l]], r_ps[:])

        # -------- phase 2: G-table build --------
        if PH >= 2:
         with tc.tile_pool(name="p2ps", bufs=2, space="PSUM") as pp2, \
                tc.tile_pool(name="p2sb", bufs=2) as ps2:
            for a0 in range(0, N_ATOMS, 512):
                ac = min(512, N_ATOMS - a0)
                nq = (ac + 127) // 128
                grows = ps2.tile([128, 4, GOFF[0]], BF16, tag="grows")
                for l in (3, 2, 1):
                    B, m, s2 = TBB[l], MSZ[l], SIDE[l] ** 2
                    ng = (KW + B - 1) // B
                    cf = ps2.tile([B * m, ng, 512], BF16, tag=f"cf{l}")
                    nc.sync.dma_start(cf[:, :, :ac],
                                      din[f'cfT_{l}'].ap()[:, :, a0:a0 + ac])
                    for g in range(ng):
                        nkb = min(B, KW - g * B)
                        gt_ps = pp2.tile([B * s2, 512], F32, tag="gt")
                        nc.tensor.matmul(gt_ps[:, :ac],
                                         lhsT=sb[f'Utab_{l}'][:],
                                         rhs=cf[:, g, :ac],
                                         start=True, stop=True)
                        gt_sb = ps2.tile([B * s2, 512], BF16, tag="gts")
                        nc.scalar.copy(gt_sb[:, :ac], gt_ps[:, :ac])
                        for q in range(nq):
                            an = min(128, ac - q * 128)
                            tp = pp2.tile([128, B * s2], BF16, tag="tp")
                            nc.tensor.transpose(
                                tp[:an, :], gt_sb[:, q * 128:q * 128 + an],
                                ident[:B * s2, :B * s2])
                            dst = grows[:an, q, :].rearrange(
                                "p (tj k) -> p tj k", k=KW)[
                                :, GOFF[l] // KW:GOFF[l] // KW + s2,
                                g * B:g * B + nkb]
                            src = tp[:an, :].rearrange(
                                "p (kb tj) -> p kb tj", kb=B)[
                                :, :nkb, :].rearrange("p kb tj -> p tj kb")
                            nc.vector.tensor_copy(dst, src)
                g0s = ps2.tile([128, 4, KW], BF16, tag="g0s")
                for q in range(nq):
                    an = min(128, ac - q * 128)
                    nc.sync.dma_start(
                        g0s[:an, q, :],
                        din['g0tab'].ap()[a0 + q * 128:a0 + q * 128 + an, :])
                    nc.sync.dma_start(
                        gtab.ap()[a0 + q * 128:a0 + q * 128 + an, :GOFF[0]],
                        grows[:an, q, :])
                    nc.sync.dma_start(
                        gtab.ap()[a0 + q * 128:a0 + q * 128 + an,
                                  GOFF[0]:GOFF[0] + KW],
                        g0s[:an, q, :])

        # -------- phase 3: edge loop --------
        coff = {3: 0, 2: 512, 1: 800, 0: 928}
        if PH >= 3:
         with tc.tile_pool(name="plps", bufs=1, space="PSUM") as poolp, \
                tc.tile_pool(name="p3ps", bufs=1, space="PSUM") as pp3, \
                tc.tile_pool(name="p3sb", bufs=2) as ps3, \
                tc.tile_pool(name="pout", bufs=1) as pout:
            for s_i in range(NCH):
                np3 = [poolp.tile([128, 400], F32, tag="pl3a", name="pl3a"),
                       poolp.tile([128, 400], F32, tag="pl3b", name="pl3b")]
                np2 = poolp.tile([128, 288], F32, tag="pl2", name="pl2")
                np10 = poolp.tile([128, 320], F32, tag="pl10", name="pl10")
                ntl = min(nt_s[s_i], int(os.environ.get("KTILES", "999")))
                for tloc in range(ntl):
                    ti = tbase[s_i] + tloc
                    first = tloc == 0
                    last = tloc == ntl - 1
                    # cvec
                    cvec = ps3.tile([128, 960], BF16, tag="cvec")
                    for l in range(4):
                        for lp in range(l + 1):
                            mlo, msz = lp * lp, 2 * lp + 1
                            dst = cvec[:, coff[l]:coff[l] + KW * MSZ[l]]\
                                .rearrange("p (k m) -> p k m", k=KW)\
                                [:, :, mlo:mlo + msz]
                            s_in = s_sb[:, ti, mlo:mlo + msz].unsqueeze(1)\
                                .broadcast_to([128, KW, msz])
                            r_in = radial_sb[:, ti, KOFF[lp] + LO[l]:
                                             KOFF[lp] + LO[l] + KW]\
                                .unsqueeze(2).broadcast_to([128, KW, msz])
                            nc.vector.tensor_tensor(out=dst, in0=s_in,
                                                    in1=r_in,
                                                    op=mybir.AluOpType.mult)
                    # transposes
                    KS = int(os.environ.get("KSTAGE", "9"))
                    cvT = {3: [], 2: [], 1: []}
                    blocks = [(3, 0, 128), (3, 128, 128), (3, 256, 128),
                              (3, 384, 128),
                              (2, 512, 72), (2, 584, 72),
                              (2, 656, 72), (2, 728, 72),
                              (1, 800, 128)]
                    for bi, (l, off, w) in enumerate(blocks) if KS >= 2 else []:
                        tp = pp3.tile([128, 128], BF16, tag="tp", bufs=1)
                        nc.tensor.transpose(tp[:w, :], cvec[:, off:off + w],
                                            ident[:])
                        piece = ps3.tile([128, 128], BF16, tag=f"cvT{bi}", name=f"cvT{bi}")
                        nc.vector.tensor_copy(piece[:w, :], tp[:w, :])
                        cvT[l].append(piece)
                    # uncouple V -> vsb (it,k) layout bf16
                    vsb3 = [ps3.tile([128, 400], BF16, tag="vs3a", name="vs3a"),
                            ps3.tile([128, 400], BF16, tag="vs3b", name="vs3b")]
                    vsb = {2: ps3.tile([128, 288], BF16, tag="vs2", name="vs2"),
                           1: ps3.tile([128, 288], BF16, tag="vs1", name="vs1")}
                    _lset = tuple(int(x) for x in os.environ.get(
                        "KLSET", "321"))
                    for l in (_lset if KS >= 3 else []):
                        nkb, m, it = VKB[l], MSZ[l], ITSZ[l]
                        if l == 3:
                            for h in range(2):
                                vp = pp3.tile([128, 400], F32, tag="vv",
                                              name="vv")
                                for qq in range(2):
                                    q = h * 2 + qq
                                    nc.tensor.matmul(
                                        vp[:, qq * 200:qq * 200 + 200],
                                        lhsT=cvT[3][q][:128, :],
                                        rhs=sb['UVbd_3'][:],
                                        start=True, stop=True)
                                if "KNOCOPY" not in os.environ:
                                    srcap = vp[:].rearrange(
                                        "p (kq kk it) -> p kq kk it",
                                        kq=2, kk=8)\
                                        .rearrange("p kq kk it -> p kq it kk")
                                    dst = vsb3[h][:].rearrange(
                                        "p (it kq kk) -> p kq it kk",
                                        kq=2, kk=8)
                                    nc.vector.tensor_copy(dst, srcap)
                        elif l == 2:
                            vp = pp3.tile([128, 288], F32, tag="vv", name="vv")
                            for q in range(4):
                                nc.tensor.matmul(
                                    vp[:, q * 72:q * 72 + 72],
                                    lhsT=cvT[2][q][:72, :],
                                    rhs=sb['UVbd_2'][:],
                                    start=True, stop=True)
                            if "KNOCOPY" not in os.environ:
                                srcap = vp[:].rearrange(
                                    "p (kq kk it) -> p kq kk it", kq=4, kk=8)\
                                    .rearrange("p kq kk it -> p kq it kk")
                                dst = vsb[2][:].rearrange(
                                    "p (it kq kk) -> p kq it kk", kq=4, kk=8)
                                nc.vector.tensor_copy(dst, srcap)
                        else:
                            vp = pp3.tile([128, 288], F32, tag="vv", name="vv")
                            nc.tensor.matmul(vp[:], lhsT=cvT[1][0][:128, :],
                                             rhs=sb['UVbd_1'][:],
                                             start=True, stop=True)
                            if "KNOCOPY" not in os.environ:
                                srcap = vp[:].rearrange(
                                    "p (kk it) -> p kk it", kk=32)\
                                    .rearrange("p kk it -> p it kk")
                                dst = vsb[1][:].rearrange(
                                    "p (it kk) -> p it kk", kk=32)
                                nc.vector.tensor_copy(dst, srcap)
                    # gather
                    G = ps3.tile([128, GROW], BF16, tag="G")
                    if PH >= 4:
                        nc.gpsimd.indirect_dma_start(
                            out=G[:], out_offset=None, in_=gtab.ap()[:, :],
                            in_offset=IndirectOffsetOnAxis(
                                ap=nbr_all[:, ti:ti + 1], axis=0))
                    else:
                        nc.gpsimd.memset(G[:], 0.0)
                    # products
                    PT = [ps3.tile([128, 1408], BF16, tag=f"PT{t}", name=f"PT{t}")
                          for t in range(5)]
                    for t in range(5) if KS >= 4 else []:
                        for h in range(2):
                            v = vsb3[h][:].rearrange(
                                "p (i t k) -> p i t k", i=5, k=16)\
                                [:, :, t, :].unsqueeze(2)\
                                .broadcast_to([128, 5, 5, 16])
                            g3 = G[:, GOFF[3]:GOFF[3] + 800].rearrange(
                                "p (t j k) -> p t j k", t=5, k=KW)\
                                [:, t, :, h * 16:h * 16 + 16].unsqueeze(1)\
                                .broadcast_to([128, 5, 5, 16])
                            o = PT[t][:, 0:800].rearrange(
                                "p (i j k) -> p i j k", i=5, k=KW)\
                                [:, :, :, h * 16:h * 16 + 16]
                            nc.vector.tensor_tensor(out=o, in0=v, in1=g3,
                                                    op=mybir.AluOpType.mult)
                    for l in (2, 1) if KS >= 4 else []:
                        for t in range(3):
                            v = vsb[l][:].rearrange(
                                "p (i t k) -> p i t k", i=3, k=KW)\
                                [:, :, t, :].unsqueeze(2)\
                                .broadcast_to([128, 3, 3, KW])
                            gl = G[:, GOFF[l]:GOFF[l] + 288].rearrange(
                                "p (t j k) -> p t j k", t=3, k=KW)\
                                [:, t, :, :].unsqueeze(1)\
                                .broadcast_to([128, 3, 3, KW])
                            o = PT[t][:, GOFF[l]:GOFF[l] + 288]\
                                .rearrange("p (i j k) -> p i j k", i=3, k=KW)
                            nc.vector.tensor_tensor(out=o, in0=v, in1=gl,
                                                    op=mybir.AluOpType.mult)
                    if KS >= 4:
                     nc.vector.tensor_tensor(
                        out=PT[0][:, 1376:1408], in0=cvec[:, 928:960],
                        in1=G[:, GOFF[0]:GOFF[0] + KW],
                        op=mybir.AluOpType.mult)
                    # segment matmuls
                    if KS < 5:
                        continue
                    lhs_ind = ind_all[:, ti, :]
                    for t in range(5):
                        for h in range(2):
                            nc.tensor.matmul(
                                np3[h][:], lhsT=lhs_ind,
                                rhs=PT[t][:, h * 400:h * 400 + 400],
                                start=(first and t == 0),
                                stop=(last and t == 4))
                    for t in range(3):
                        nc.tensor.matmul(np2[:], lhsT=lhs_ind,
                                         rhs=PT[t][:, 800:1088],
                                         start=(first and t == 0),
                                         stop=(last and t == 2))
                    nc.tensor.matmul(np10[:], lhsT=lhs_ind,
                                     rhs=PT[0][:, 1088:1408],
                                     start=first, stop=False)
                    for t in (1, 2):
                        nc.tensor.matmul(np10[:, :288], lhsT=lhs_ind,
                                         rhs=PT[t][:, 1088:1376],
                                         start=False, stop=(last and t == 2))
                # ---- chunk epilogue ----
                if int(os.environ.get("KSTAGE", "9")) < 5:
                    continue
                pooled = pout.tile([128, PTOT], BF16, tag="pooled")
                nc.scalar.copy(pooled[:, 0:400], np3[0][:])
                nc.scalar.copy(pooled[:, 400:800], np3[1][:])
                nc.scalar.copy(pooled[:, 800:1088], np2[:])
                nc.scalar.copy(pooled[:, 1088:1408], np10[:])
                pieces = pout.tile([128, NPIECE, 128], BF16, tag="pieces")
                for p in range(NPIECE):
                    tp = pp3.tile([128, 128], BF16, tag="tp", bufs=1)
                    nc.tensor.transpose(tp[:], pooled[:, p * 128:p * 128 + 128],
                                        ident[:])
                    nc.vector.tensor_copy(pieces[:, p, :], tp[:])
                for l in range(4):
                    ncol = (2 * l + 1) * KMAX[l]
                    fo = pout.tile([128, 960], F32, tag="fo")
                    nc.sync.dma_start(
                        fo[:, :ncol],
                        din[f'featown_{l}'].ap()[s_i * 128:s_i * 128 + 128, :])
                    for c0 in range(0, ncol, 128):
                        cw = min(128, ncol - c0)
                        ops = pp3.tile([128, 128], F32, tag="ops")
                        for p in range(NPIECE):
                            nc.tensor.matmul(
                                ops[:cw, :],
                                lhsT=sb[f'WU_{l}'][:, p, c0:c0 + cw],
                                rhs=pieces[:, p, :],
                                start=(p == 0), stop=(p == NPIECE - 1))
                        osb = pout.tile([128, 128], BF16, tag="osb")
                        nc.scalar.copy(osb[:cw, :], ops[:cw, :])
                        tp2 = pp3.tile([128, 128], BF16, tag="tp", bufs=1)
                        nc.tensor.transpose(tp2[:, :cw], osb[:cw, :],
                                            ident[:cw, :cw])
                        ofin = pout.tile([128, 128], F32, tag="ofin", bufs=2)
                        nc.vector.tensor_add(out=ofin[:, :cw],
                                             in0=tp2[:, :cw],
                                             in1=fo[:, c0:c0 + cw])
                        nc.sync.dma_start(
                            douts[l].ap()[s_i * 128:s_i * 128 + 128,
                                          c0:c0 + cw], ofin[:, :cw])
        ctx.close()
    nc.compile()
    return nc


def kernel(**inputs):
    per_core, rep, meta = _host_prep(inputs)
    nc = build_program(meta)
    in_maps = []
    for c in range(NC_):
        m = dict(per_core[c])
        m.update(rep)
        in_maps.append(m)
    res = run_bass_kernel_spmd(nc, in_maps, list(range(NC_)))
    outs = []
    abnd = meta['abnd']
    for l in range(4):
        full = np.zeros((N_ATOMS, 2 * l + 1, KMAX[l]), np.float32)
        for c in range(NC_):
            a0, a1 = abnd[c], abnd[c + 1]
            full[a0:a1] = res.results[c][f'out_{l}'][:a1 - a0].reshape(
                a1 - a0, 2 * l + 1, KMAX[l])
        outs.append(full)
    return tuple(outs)


# revision 21
# speedup vs baseline: 1.1391x; 1.1391x over previous
"""Trainium2 Bass kernel for nn_EquivariantMessagePasser (8-core SPMD).

Strategy: edges sorted+sharded by center atom (segment-sum is core-local via
per-tile indicator matmuls into accumulating PSUM); feats replicated; per-atom
uncoupled-feature table built on device in DRAM and gathered per edge via
indirect DMA; couple-back + output linear folded into one host-precomputed
weight (WU = U x Wl).
"""
import sys

sys.path.insert(0, "/opt/trn_rl_repo")

from contextlib import ExitStack

import numpy as np
import ml_dtypes

import concourse.bass as bass
from concourse import bacc, mybir
from concourse.bass import IndirectOffsetOnAxis
from concourse.tile import TileContext
from concourse.bass_utils import run_bass_kernel_spmd
from concourse.masks import make_identity

F32 = mybir.dt.float32
BF16 = mybir.dt.bfloat16
I32 = mybir.dt.int32
bf = ml_dtypes.bfloat16

NMAX = [8, 6, 4, 2]
KMAX = [128, 96, 64, 32]
PADDED_L = [0, 2, 2, 4]
SIDE = [1, 3, 3, 5]
MSZ = [1, 4, 9, 16]
ITSZ = [1, 9, 9, 25]
N_ATOMS = 2500
HIDDEN = 64
NC_ = 8
KW = 32
LO = [96, 64, 32, 0]
KOFF = [0, 128, 224, 288]         # radial region offsets (cumsum KMAX)

# row layout: [l3 (i/t,j,k)=800 | l2 288 | l1 288 | l0 32] = 1408
GOFF = {3: 0, 2: 800, 1: 1088, 0: 1376}
GROW = 1408
PTOT = 1408
NPIECE = 11

VKB = {3: 8, 2: 8, 1: 32}         # k-channels per uncouple-V matmul
TBB = {3: 5, 2: 14, 1: 14}        # k-channels per table-build block


def _uflat(U):
    side = U.shape[0]
    return U.reshape(side * side, side * side).T.copy()  # [m, (i,j)]


def _blockdiag(mat, B):
    m, n = mat.shape
    out = np.zeros((B * m, B * n), mat.dtype)
    for b in range(B):
        out[b * m:(b + 1) * m, b * n:(b + 1) * n] = mat
    return out


def _cfeat(feats, l):
    return np.concatenate(
        [feats[lp][:, :, LO[l]:LO[l] + KW] for lp in range(l + 1)], axis=1)


def _build_wu(U, Wl):
    u0 = float(np.asarray(U[0]).reshape(-1)[0])
    wu = []
    for l in range(4):
        ncol = (2 * l + 1) * KMAX[l]
        M = np.zeros((PTOT, ncol), np.float32)
        for lch in range(l, 4):
            s = SIDE[lch]
            uf = np.asarray(U[PADDED_L[lch]], np.float32)
            koff = (lch - l) * KW
            for i in range(s):
                for j in range(s):
                    for mloc in range(2 * l + 1):
                        uv = float(uf[i, j, l * l + mloc])
                        if lch == 0:
                            uv *= u0 * u0
                        r0 = GOFF[lch] + (i * s + j) * KW
                        M[r0:r0 + KW, mloc * KMAX[l]:(mloc + 1) * KMAX[l]] += \
                            uv * Wl[l][koff:koff + KW, :]
        wu.append(M)
    return wu


def _host_prep(inp):
    rb = [np.asarray(inp[f'radial_basis_{l}'], np.float32) for l in range(4)]
    sph = [np.asarray(inp[f'spherical_harmonics_{l}'], np.float32)
           for l in range(4)]
    feats = [np.asarray(inp[f'features_{l}'], np.float32) for l in range(4)]
    centers = np.asarray(inp['centers'])
    neighbors = np.asarray(inp['neighbors'])
    U = {L: np.asarray(inp[f'U{L}'], np.float32) for L in (0, 2, 4)}
    Wr1 = [np.asarray(inp[f'Wr1_{l}'], np.float32) for l in range(4)]
    Wr2 = [np.asarray(inp[f'Wr2_{l}'], np.float32) for l in range(4)]
    Wl = [np.asarray(inp[f'Wl_{l}'], np.float32) for l in range(4)]

    order = np.argsort(centers, kind='stable')
    c_sorted = centers[order]
    abnd = [round(c * N_ATOMS / NC_) for c in range(NC_ + 1)]
    starts = np.searchsorted(c_sorted, np.arange(N_ATOMS + 1))
    core_chunks = []
    nch_max = 0
    for c in range(NC_):
        a0c, a1c = abnd[c], abnd[c + 1]
        chunks = []
        a = a0c
        while a < a1c:
            na = min(128, a1c - a)
            e0, e1 = int(starts[a]), int(starts[a + na])
            chunks.append((a, na, e0, e1 - e0))
            a += na
        core_chunks.append(chunks)
        nch_max = max(nch_max, len(chunks))
    NCH = nch_max
    for c in range(NC_):
        while len(core_chunks[c]) < NCH:
            core_chunks[c].append((abnd[c + 1], 0, 0, 0))
    nt_s = [max((core_chunks[c][s][3] + 127) // 128 for c in range(NC_))
            for s in range(NCH)]
    NT = sum(nt_s)
    EP = NT * 128
    tbase = np.cumsum([0] + nt_s)

    sph_cat = np.concatenate(sph, axis=1)

    per_core = []
    for c in range(NC_):
        eidx = np.zeros(EP, np.int64)
        valid = np.zeros(EP, np.float32)
        ind = np.zeros((NT, 128, 128), np.float32)
        for s_i, (a0, na, e0, ne) in enumerate(core_chunks[c]):
            pos0 = int(tbase[s_i]) * 128
            idx = order[e0:e0 + ne]
            eidx[pos0:pos0 + ne] = idx
            valid[pos0:pos0 + ne] = 1.0
            loc = c_sorted[e0:e0 + ne] - a0
            rows = np.arange(pos0, pos0 + ne)
            ind[rows // 128, rows % 128, loc] = 1.0
        d = {}
        for l in range(4):
            d[f'rbT_{l}'] = np.ascontiguousarray(
                (rb[l][eidx] * valid[:, None]).T).astype(bf)
        d['s_mat'] = (sph_cat[eidx] * valid[:, None]).astype(np.float32)
        d['nbr'] = np.ascontiguousarray(
            (neighbors[eidx] * valid.astype(np.int64)).astype(np.int32)
            [:, None])
        d['ind'] = ind.reshape(NT * 128, 128).astype(bf)
        a0c, a1c = abnd[c], abnd[c + 1]
        for l in range(4):
            fo = np.zeros((NCH * 128, (2 * l + 1) * KMAX[l]), np.float32)
            fo[:a1c - a0c] = feats[l][a0c:a1c].reshape(a1c - a0c, -1)
            d[f'featown_{l}'] = fo
        per_core.append(d)

    rep = {}
    for l in (1, 2, 3):
        B, m = TBB[l], MSZ[l]
        cf = _cfeat(feats, l)
        ng = (KW + B - 1) // B
        t = np.zeros((B * m, ng, N_ATOMS), np.float32)
        for g in range(ng):
            for kb in range(B):
                k = g * B + kb
                if k < KW:
                    t[kb * m:(kb + 1) * m, g, :] = cf[:, :, k].T
        rep[f'cfT_{l}'] = t.astype(bf)
        uf = _uflat(U[PADDED_L[l]])[:m, :]
        rep[f'Utab_{l}'] = _blockdiag(uf, B).astype(bf)
        rep[f'UVbd_{l}'] = _blockdiag(uf, VKB[l]).astype(bf)
    rep['g0tab'] = np.ascontiguousarray(feats[0][:, 0, 96:128]).astype(bf)
    wu = _build_wu(U, Wl)
    for l in range(4):
        rep[f'WU_{l}'] = wu[l].astype(bf)
        rep[f'Wr1_{l}'] = Wr1[l].astype(bf)
        rep[f'Wr2_{l}'] = Wr2[l].astype(bf)

    meta = dict(NT=NT, NCH=NCH, nt_s=nt_s, tbase=[int(x) for x in tbase],
                abnd=abnd, EP=EP)
    return per_core, rep, meta


def build_program(meta):
    import os
    PH = int(os.environ.get("KPHASE", "4"))
    NT, NCH, EP = meta['NT'], meta['NCH'], meta['EP']
    nt_s, tbase = meta['nt_s'], meta['tbase']

    nc = bacc.Bacc("TRN2", target_bir_lowering=False, debug=False,
                   num_devices=NC_)
    ctx = ExitStack()

    din = {}

    def dri(name, shape, dt):
        din[name] = nc.dram_tensor(name, shape, dt, kind="ExternalInput")

    for l in range(4):
        dri(f'rbT_{l}', [NMAX[l], EP], BF16)
        dri(f'featown_{l}', [NCH * 128, (2 * l + 1) * KMAX[l]], F32)
        dri(f'WU_{l}', [PTOT, (2 * l + 1) * KMAX[l]], BF16)
        dri(f'Wr1_{l}', [NMAX[l], HIDDEN], BF16)
        dri(f'Wr2_{l}', [HIDDEN, KMAX[l]], BF16)
    dri('s_mat', [EP, 16], F32)
    dri('nbr', [EP, 1], I32)
    dri('ind', [NT * 128, 128], BF16)
    for l in (1, 2, 3):
        B, m = TBB[l], MSZ[l]
        ng = (KW + B - 1) // B
        dri(f'cfT_{l}', [B * m, ng, N_ATOMS], BF16)
        dri(f'Utab_{l}', [B * m, B * SIDE[l] ** 2], BF16)
        dri(f'UVbd_{l}', [VKB[l] * m, VKB[l] * ITSZ[l]], BF16)
    dri('g0tab', [N_ATOMS, KW], BF16)
    douts = [nc.dram_tensor(f'out_{l}', [NCH * 128, (2 * l + 1) * KMAX[l]],
                            F32, kind="ExternalOutput") for l in range(4)]
    gtab = nc.dram_tensor('gtab', [N_ATOMS, GROW], BF16)

    with TileContext(nc) as tc:
        cpool = ctx.enter_context(tc.tile_pool(name="const", bufs=1))
        ident = cpool.tile([128, 128], BF16)
        make_identity(nc, ident[:])

        sb = {}
        for name in ('Utab_1', 'Utab_2', 'Utab_3', 'UVbd_1', 'UVbd_2',
                     'UVbd_3', 'Wr1_0', 'Wr1_1', 'Wr1_2', 'Wr1_3',
                     'Wr2_0', 'Wr2_1', 'Wr2_2', 'Wr2_3'):
            t = din[name]
            sb[name] = cpool.tile(list(t.shape), t.dtype, name=name)
            nc.sync.dma_start(sb[name][:], t.ap())
        for l in range(4):
            t = din[f'WU_{l}']
            w = cpool.tile([128, NPIECE, t.shape[1]], BF16, name=f'wu{l}')
            sb[f'WU_{l}'] = w
            nc.sync.dma_start(w[:],
                              t.ap().rearrange("(p q) c -> q p c", q=128))
        s_sb = cpool.tile([128, NT, 16], F32)
        nc.sync.dma_start(
            s_sb[:], din['s_mat'].ap().rearrange("(t q) m -> q t m", q=128))
        nbr_all = cpool.tile([128, NT], I32)
        nc.sync.dma_start(
            nbr_all[:], din['nbr'].ap().rearrange("(t q) one -> q (t one)",
                                                  q=128))
        ind_all = cpool.tile([128, NT, 128], BF16)
        nc.sync.dma_start(
            ind_all[:], din['ind'].ap().rearrange("(t q) a -> q t a", q=128))
        rbT_sb = {}
        for l in range(4):
            rbT_sb[l] = cpool.tile([NMAX[l], EP], BF16, name=f'rbt{l}')
            nc.sync.dma_start(rbT_sb[l][:], din[f'rbT_{l}'].ap())

        # -------- phase 1: radial MLP --------
        radial_sb = cpool.tile([128, NT, 320], F32)
        if PH >= 1:
         with tc.tile_pool(name="p1ps", bufs=2, space="PSUM") as pp1, \
                tc.tile_pool(name="p1sb", bufs=2) as ps1:
            for l in range(4):
                for t0 in range(0, NT, 4):
                    nt4 = min(4, NT - t0)
                    ec = nt4 * 128
                    h_ps = pp1.tile([HIDDEN, 512], F32, tag="h")
                    nc.tensor.matmul(h_ps[:, :ec], lhsT=sb[f'Wr1_{l}'][:],
                                     rhs=rbT_sb[l][:, t0 * 128:t0 * 128 + ec],
                                     start=True, stop=True)
                    h_sg = ps1.tile([HIDDEN, 512], F32, tag="hsg")
                    nc.scalar.activation(h_sg[:, :ec], h_ps[:, :ec],
                                         mybir.ActivationFunctionType.Sigmoid)
                    h_sb = ps1.tile([HIDDEN, 512], BF16, tag="hs")
                    nc.vector.tensor_tensor(out=h_sb[:, :ec],
                                            in0=h_ps[:, :ec],
                                            in1=h_sg[:, :ec],
                                            op=mybir.AluOpType.mult)
                    for ti in range(nt4):
                        r_ps = pp1.tile([128, KMAX[l]], F32, tag="r")
                        nc.tensor.matmul(r_ps[:],
                                         lhsT=h_sb[:, ti * 128:ti * 128 + 128],
                                         rhs=sb[f'Wr2_{l}'][:],
                                         start=True, stop=True)
                        nc.scalar.copy(
                            radial_sb[:, t0 + ti,
                                      KOFF[l]:KOFF[l] + KMAX[l]], r_ps[:])

        # -------- phase 2: G-table build --------
        if PH >= 2:
         with tc.tile_pool(name="p2ps", bufs=2, space="PSUM") as pp2, \
                tc.tile_pool(name="p2sb", bufs=2) as ps2:
            for a0 in range(0, N_ATOMS, 512):
                ac = min(512, N_ATOMS - a0)
                nq = (ac + 127) // 128
                grows = ps2.tile([128, 4, GOFF[0]], BF16, tag="grows")
                for l in (3, 2, 1):
                    B, m, s2 = TBB[l], MSZ[l], SIDE[l] ** 2
                    ng = (KW + B - 1) // B
                    cf = ps2.tile([B * m, ng, 512], BF16, tag=f"cf{l}")
                    nc.sync.dma_start(cf[:, :, :ac],
                                      din[f'cfT_{l}'].ap()[:, :, a0:a0 + ac])
                    for g in range(ng):
                        nkb = min(B, KW - g * B)
                        gt_ps = pp2.tile([B * s2, 512], F32, tag="gt")
                        nc.tensor.matmul(gt_ps[:, :ac],
                                         lhsT=sb[f'Utab_{l}'][:],
                                         rhs=cf[:, g, :ac],
                                         start=True, stop=True)
                        gt_sb = ps2.tile([B * s2, 512], BF16, tag="gts")
                        nc.scalar.copy(gt_sb[:, :ac], gt_ps[:, :ac])
                        for q in range(nq):
                            an = min(128, ac - q * 128)
                            tp = pp2.tile([128, B * s2], BF16, tag="tp")
                            nc.tensor.transpose(
                                tp[:an, :], gt_sb[:, q * 128:q * 128 + an],
                                ident[:B * s2, :B * s2])
                            dst = grows[:an, q, :].rearrange(
                                "p (tj k) -> p tj k", k=KW)[
                                :, GOFF[l] // KW:GOFF[l] // KW + s2,
                                g * B:g * B + nkb]
                            src = tp[:an, :].rearrange(
                                "p (kb tj) -> p kb tj", kb=B)[
                                :, :nkb, :].rearrange("p kb tj -> p tj kb")
                            nc.vector.tensor_copy(dst, src)
                g0s = ps2.tile([128, 4, KW], BF16, tag="g0s")
                for q in range(nq):
                    an = min(128, ac - q * 128)
                    nc.sync.dma_start(
                        g0s[:an, q, :],
                        din['g0tab'].ap()[a0 + q * 128:a0 + q * 128 + an, :])
                    nc.sync.dma_start(
                        gtab.ap()[a0 + q * 128:a0 + q * 128 + an, :GOFF[0]],
                        grows[:an, q, :])
                    nc.sync.dma_start(
                        gtab.ap()[a0 + q * 128:a0 + q * 128 + an,
                                  GOFF[0]:GOFF[0] + KW],
                        g0s[:an, q, :])

        # -------- phase 3: edge loop --------
        coff = {3: 0, 2: 512, 1: 800, 0: 928}
        if PH >= 3:
         with tc.tile_pool(name="plps", bufs=1, space="PSUM") as poolp, \
                tc.tile_pool(name="p3ps", bufs=1, space="PSUM") as pp3, \
                tc.tile_pool(name="p3sb", bufs=2) as ps3, \
                tc.tile_pool(name="pout", bufs=1) as pout:
            for s_i in range(NCH):
                np3 = [poolp.tile([128, 400], F32, tag="pl3a", name="pl3a"),
                       poolp.tile([128, 400], F32, tag="pl3b", name="pl3b")]
                np2 = poolp.tile([128, 288], F32, tag="pl2", name="pl2")
                np10 = poolp.tile([128, 320], F32, tag="pl10", name="pl10")
                ntl = min(nt_s[s_i], int(os.environ.get("KTILES", "999")))
                for tloc in range(ntl):
                    ti = tbase[s_i] + tloc
                    first = tloc == 0
                    last = tloc == ntl - 1
                    # cvec
                    cvec = ps3.tile([128, 960], BF16, tag="cvec")
                    for l in range(4):
                        for lp in range(l + 1):
                            mlo, msz = lp * lp, 2 * lp + 1
                            dst = cvec[:, coff[l]:coff[l] + KW * MSZ[l]]\
                                .rearrange("p (k m) -> p k m", k=KW)\
                                [:, :, mlo:mlo + msz]
                            s_in = s_sb[:, ti, mlo:mlo + msz].unsqueeze(1)\
                                .broadcast_to([128, KW, msz])
                            r_in = radial_sb[:, ti, KOFF[lp] + LO[l]:
                                             KOFF[lp] + LO[l] + KW]\
                                .unsqueeze(2).broadcast_to([128, KW, msz])
                            nc.vector.tensor_tensor(out=dst, in0=s_in,
                                                    in1=r_in,
                                                    op=mybir.AluOpType.mult)
                    # transposes
                    KS = int(os.environ.get("KSTAGE", "9"))
                    cvT = {3: [], 2: [], 1: []}
                    blocks = [(3, 0, 128), (3, 128, 128), (3, 256, 128),
                              (3, 384, 128),
                              (2, 512, 72), (2, 584, 72),
                              (2, 656, 72), (2, 728, 72),
                              (1, 800, 128)]
                    for bi, (l, off, w) in enumerate(blocks) if KS >= 2 else []:
                        tp = pp3.tile([128, 128], BF16, tag="tp", bufs=1)
                        nc.tensor.transpose(tp[:w, :], cvec[:, off:off + w],
                                            ident[:])
                        piece = ps3.tile([128, 128], BF16, tag=f"cvT{bi}", name=f"cvT{bi}")
                        nc.scalar.copy(piece[:w, :], tp[:w, :])
                        cvT[l].append(piece)
                    # uncouple V -> vsb (it,k) layout bf16
                    vsb3 = ps3.tile([128, 800], BF16, tag="vs3", name="vs3")
                    vsb = {2: ps3.tile([128, 288], BF16, tag="vs2", name="vs2"),
                           1: ps3.tile([128, 288], BF16, tag="vs1", name="vs1")}
                    _lset = tuple(int(x) for x in os.environ.get(
                        "KLSET", "321"))
                    for l in (_lset if KS >= 3 else []):
                        nkb, m, it = VKB[l], MSZ[l], ITSZ[l]
                        if l == 3:
                            for h in range(2):
                                vp = pp3.tile([128, 400], F32, tag="vv",
                                              name="vv")
                                for qq in range(2):
                                    q = h * 2 + qq
                                    nc.tensor.matmul(
                                        vp[:, qq * 200:qq * 200 + 200],
                                        lhsT=cvT[3][q][:128, :],
                                        rhs=sb['UVbd_3'][:],
                                        start=True, stop=True)
                                if "KNOCOPY" not in os.environ:
                                    srcap = vp[:].rearrange(
                                        "p (kq kk it) -> p kq kk it",
                                        kq=2, kk=8)\
                                        .rearrange("p kq kk it -> p kq it kk")
                                    dst = vsb3[:].rearrange(
                                        "p (it k) -> p it k", k=KW)\
                                        [:, :, h * 16:h * 16 + 16].rearrange(
                                        "p it (kq kk) -> p kq it kk", kq=2)
                                    nc.scalar.copy(dst, srcap)
                        elif l == 2:
                            vp = pp3.tile([128, 288], F32, tag="vv", name="vv")
                            for q in range(4):
                                nc.tensor.matmul(
                                    vp[:, q * 72:q * 72 + 72],
                                    lhsT=cvT[2][q][:72, :],
                                    rhs=sb['UVbd_2'][:],
                                    start=True, stop=True)
                            if "KNOCOPY" not in os.environ:
                                srcap = vp[:].rearrange(
                                    "p (kq kk it) -> p kq kk it", kq=4, kk=8)\
                                    .rearrange("p kq kk it -> p kq it kk")
                                dst = vsb[2][:].rearrange(
                                    "p (it kq kk) -> p kq it kk", kq=4, kk=8)
                                nc.scalar.copy(dst, srcap)
                        else:
                            vp = pp3.tile([128, 288], F32, tag="vv", name="vv")
                            nc.tensor.matmul(vp[:], lhsT=cvT[1][0][:128, :],
                                             rhs=sb['UVbd_1'][:],
                                             start=True, stop=True)
                            if "KNOCOPY" not in os.environ:
                                srcap = vp[:].rearrange(
                                    "p (kk it) -> p kk it", kk=32)\
                                    .rearrange("p kk it -> p it kk")
                                dst = vsb[1][:].rearrange(
                                    "p (it kk) -> p it kk", kk=32)
                                nc.scalar.copy(dst, srcap)
                    # gather
                    G = ps3.tile([128, GROW], BF16, tag="G")
                    if PH >= 4:
                        nc.gpsimd.indirect_dma_start(
                            out=G[:], out_offset=None, in_=gtab.ap()[:, :],
                            in_offset=IndirectOffsetOnAxis(
                                ap=nbr_all[:, ti:ti + 1], axis=0))
                    else:
                        nc.gpsimd.memset(G[:], 0.0)
                    # products
                    PT = [ps3.tile([128, 1408], BF16, tag=f"PT{t}", name=f"PT{t}")
                          for t in range(5)]
                    for t in range(5) if KS >= 4 else []:
                        v = vsb3[:].rearrange(
                            "p (i t k) -> p i t k", i=5, k=KW)\
                            [:, :, t, :].unsqueeze(2)\
                            .broadcast_to([128, 5, 5, KW])
                        g3 = G[:, GOFF[3]:GOFF[3] + 800].rearrange(
                            "p (t j k) -> p t j k", t=5, k=KW)\
                            [:, t, :, :].unsqueeze(1)\
                            .broadcast_to([128, 5, 5, KW])
                        o = PT[t][:, 0:800].rearrange(
                            "p (i j k) -> p i j k", i=5, k=KW)
                        nc.vector.tensor_tensor(out=o, in0=v, in1=g3,
                                                op=mybir.AluOpType.mult)
                    for l in (2, 1) if KS >= 4 else []:
                        for t in range(3):
                            v = vsb[l][:].rearrange(
                                "p (i t k) -> p i t k", i=3, k=KW)\
                                [:, :, t, :].unsqueeze(2)\
                                .broadcast_to([128, 3, 3, KW])
                            gl = G[:, GOFF[l]:GOFF[l] + 288].rearrange(
                                "p (t j k) -> p t j k", t=3, k=KW)\
                                [:, t, :, :].unsqueeze(1)\
                                .broadcast_to([128, 3, 3, KW])
                            o = PT[t][:, GOFF[l]:GOFF[l] + 288]\
                                .rearrange("p (i j k) -> p i j k", i=3, k=KW)
                            nc.vector.tensor_tensor(out=o, in0=v, in1=gl,
                                                    op=mybir.AluOpType.mult)
                    if KS >= 4:
                     nc.vector.tensor_tensor(
                        out=PT[0][:, 1376:1408], in0=cvec[:, 928:960],
                        in1=G[:, GOFF[0]:GOFF[0] + KW],
                        op=mybir.AluOpType.mult)
                    # segment matmuls
                    if KS < 5:
                        continue
                    lhs_ind = ind_all[:, ti, :]
                    for t in range(5):
                        for h in range(2):
                            nc.tensor.matmul(
                                np3[h][:], lhsT=lhs_ind,
                                rhs=PT[t][:, h * 400:h * 400 + 400],
                                start=(first and t == 0),
                                stop=(last and t == 4))
                    for t in range(3):
                        nc.tensor.matmul(np2[:], lhsT=lhs_ind,
                                         rhs=PT[t][:, 800:1088],
                                         start=(first and t == 0),
                                         stop=(last and t == 2))
                    nc.tensor.matmul(np10[:], lhsT=lhs_ind,
                                     rhs=PT[0][:, 1088:1408],
                                     start=first, stop=False)
                    for t in (1, 2):
                        nc.tensor.matmul(np10[:, :288], lhsT=lhs_ind,
                                         rhs=PT[t][:, 1088:1376],
                                         start=False, stop=(last and t == 2))
                # ---- chunk epilogue ----
                if int(os.environ.get("KSTAGE", "9")) < 5:
                    continue
                pooled = pout.tile([128, PTOT], BF16, tag="pooled")
                nc.scalar.copy(pooled[:, 0:400], np3[0][:])
                nc.scalar.copy(pooled[:, 400:800], np3[1][:])
                nc.scalar.copy(pooled[:, 800:1088], np2[:])
                nc.scalar.copy(pooled[:, 1088:1408], np10[:])
                pieces = pout.tile([128, NPIECE, 128], BF16, tag="pieces")
                for p in range(NPIECE):
                    tp = pp3.tile([128, 128], BF16, tag="tp", bufs=1)
                    nc.tensor.transpose(tp[:], pooled[:, p * 128:p * 128 + 128],
                                        ident[:])
                    nc.scalar.copy(pieces[:, p, :], tp[:])
                for l in range(4):
                    ncol = (2 * l + 1) * KMAX[l]
                    fo = pout.tile([128, 960], F32, tag="fo")
                    nc.sync.dma_start(
                        fo[:, :ncol],
                        din[f'featown_{l}'].ap()[s_i * 128:s_i * 128 + 128, :])
                    for c0 in range(0, ncol, 128):
                        cw = min(128, ncol - c0)
                        ops = pp3.tile([128, 128], F32, tag="ops")
                        for p in range(NPIECE):
                            nc.tensor.matmul(
                                ops[:cw, :],
                                lhsT=sb[f'WU_{l}'][:, p, c0:c0 + cw],
                                rhs=pieces[:, p, :],
                                start=(p == 0), stop=(p == NPIECE - 1))
                        osb = pout.tile([128, 128], BF16, tag="osb")
                        nc.scalar.copy(osb[:cw, :], ops[:cw, :])
                        tp2 = pp3.tile([128, 128], BF16, tag="tp", bufs=1)
                        nc.tensor.transpose(tp2[:, :cw], osb[:cw, :],
                                            ident[:cw, :cw])
                        ofin = pout.tile([128, 128], F32, tag="ofin", bufs=2)
                        nc.vector.tensor_add(out=ofin[:, :cw],
                                             in0=tp2[:, :cw],
                                             in1=fo[:, c0:c0 + cw])
                        nc.sync.dma_start(
                            douts[l].ap()[s_i * 128:s_i * 128 + 128,
                                          c0:c0 + cw], ofin[:, :cw])
        ctx.close()
    nc.compile()
    return nc


def kernel(**inputs):
    per_core, rep, meta = _host_prep(inputs)
    nc = build_program(meta)
    in_maps = []
    for c in range(NC_):
        m = dict(per_core[c])
        m.update(rep)
        in_maps.append(m)
    res = run_bass_kernel_spmd(nc, in_maps, list(range(NC_)))
    outs = []
    abnd = meta['abnd']
    for l in range(4):
        full = np.zeros((N_ATOMS, 2 * l + 1, KMAX[l]), np.float32)
        for c in range(NC_):
            a0, a1 = abnd[c], abnd[c + 1]
            full[a0:a1] = res.results[c][f'out_{l}'][:a1 - a0].reshape(
                a1 - a0, 2 * l + 1, KMAX[l])
        outs.append(full)
    return tuple(outs)


# revision 23
# speedup vs baseline: 1.2084x; 1.0609x over previous
"""Trainium2 Bass kernel for nn_EquivariantMessagePasser (8-core SPMD).

Strategy: edges sorted+sharded by center atom (segment-sum is core-local via
per-tile indicator matmuls into accumulating PSUM); feats replicated; per-atom
uncoupled-feature table built on device in DRAM and gathered per edge via
indirect DMA; couple-back + output linear folded into one host-precomputed
weight (WU = U x Wl).
"""
import sys

sys.path.insert(0, "/opt/trn_rl_repo")

from contextlib import ExitStack

import numpy as np
import ml_dtypes

import concourse.bass as bass
from concourse import bacc, mybir
from concourse.bass import IndirectOffsetOnAxis
from concourse.tile import TileContext
from concourse.bass_utils import run_bass_kernel_spmd
from concourse.masks import make_identity

F32 = mybir.dt.float32
BF16 = mybir.dt.bfloat16
I32 = mybir.dt.int32
bf = ml_dtypes.bfloat16

NMAX = [8, 6, 4, 2]
KMAX = [128, 96, 64, 32]
PADDED_L = [0, 2, 2, 4]
SIDE = [1, 3, 3, 5]
MSZ = [1, 4, 9, 16]
ITSZ = [1, 9, 9, 25]
N_ATOMS = 2500
HIDDEN = 64
NC_ = 8
KW = 32
LO = [96, 64, 32, 0]
KOFF = [0, 128, 224, 288]         # radial region offsets (cumsum KMAX)

# row layout: [l3 (i/t,j,k)=800 | l2 288 | l1 288 | l0 32] = 1408
GOFF = {3: 0, 2: 800, 1: 1088, 0: 1376}
GROW = 1408
PTOT = 1408
NPIECE = 11

VKB = {3: 8, 2: 8, 1: 32}         # k-channels per uncouple-V matmul
TBB = {3: 5, 2: 14, 1: 14}        # k-channels per table-build block


def _uflat(U):
    side = U.shape[0]
    return U.reshape(side * side, side * side).T.copy()  # [m, (i,j)]


def _blockdiag(mat, B):
    m, n = mat.shape
    out = np.zeros((B * m, B * n), mat.dtype)
    for b in range(B):
        out[b * m:(b + 1) * m, b * n:(b + 1) * n] = mat
    return out


def _cfeat(feats, l):
    return np.concatenate(
        [feats[lp][:, :, LO[l]:LO[l] + KW] for lp in range(l + 1)], axis=1)


def _build_wu(U, Wl):
    u0 = float(np.asarray(U[0]).reshape(-1)[0])
    wu = []
    for l in range(4):
        ncol = (2 * l + 1) * KMAX[l]
        M = np.zeros((PTOT, ncol), np.float32)
        for lch in range(l, 4):
            s = SIDE[lch]
            uf = np.asarray(U[PADDED_L[lch]], np.float32)
            koff = (lch - l) * KW
            for i in range(s):
                for j in range(s):
                    for mloc in range(2 * l + 1):
                        uv = float(uf[i, j, l * l + mloc])
                        if lch == 0:
                            uv *= u0 * u0
                        r0 = GOFF[lch] + (i * s + j) * KW
                        M[r0:r0 + KW, mloc * KMAX[l]:(mloc + 1) * KMAX[l]] += \
                            uv * Wl[l][koff:koff + KW, :]
        wu.append(M)
    return wu


def _host_prep(inp):
    rb = [np.asarray(inp[f'radial_basis_{l}'], np.float32) for l in range(4)]
    sph = [np.asarray(inp[f'spherical_harmonics_{l}'], np.float32)
           for l in range(4)]
    feats = [np.asarray(inp[f'features_{l}'], np.float32) for l in range(4)]
    centers = np.asarray(inp['centers'])
    neighbors = np.asarray(inp['neighbors'])
    U = {L: np.asarray(inp[f'U{L}'], np.float32) for L in (0, 2, 4)}
    Wr1 = [np.asarray(inp[f'Wr1_{l}'], np.float32) for l in range(4)]
    Wr2 = [np.asarray(inp[f'Wr2_{l}'], np.float32) for l in range(4)]
    Wl = [np.asarray(inp[f'Wl_{l}'], np.float32) for l in range(4)]

    order = np.argsort(centers, kind='stable')
    c_sorted = centers[order]
    abnd = [round(c * N_ATOMS / NC_) for c in range(NC_ + 1)]
    starts = np.searchsorted(c_sorted, np.arange(N_ATOMS + 1))
    core_chunks = []
    nch_max = 0
    for c in range(NC_):
        a0c, a1c = abnd[c], abnd[c + 1]
        chunks = []
        a = a0c
        while a < a1c:
            na = min(128, a1c - a)
            e0, e1 = int(starts[a]), int(starts[a + na])
            chunks.append((a, na, e0, e1 - e0))
            a += na
        core_chunks.append(chunks)
        nch_max = max(nch_max, len(chunks))
    NCH = nch_max
    for c in range(NC_):
        while len(core_chunks[c]) < NCH:
            core_chunks[c].append((abnd[c + 1], 0, 0, 0))
    nt_s = [max((core_chunks[c][s][3] + 127) // 128 for c in range(NC_))
            for s in range(NCH)]
    NT = sum(nt_s)
    EP = NT * 128
    tbase = np.cumsum([0] + nt_s)

    sph_cat = np.concatenate(sph, axis=1)

    per_core = []
    for c in range(NC_):
        eidx = np.zeros(EP, np.int64)
        valid = np.zeros(EP, np.float32)
        ind = np.zeros((NT, 128, 128), np.float32)
        for s_i, (a0, na, e0, ne) in enumerate(core_chunks[c]):
            pos0 = int(tbase[s_i]) * 128
            idx = order[e0:e0 + ne]
            eidx[pos0:pos0 + ne] = idx
            valid[pos0:pos0 + ne] = 1.0
            loc = c_sorted[e0:e0 + ne] - a0
            rows = np.arange(pos0, pos0 + ne)
            ind[rows // 128, rows % 128, loc] = 1.0
        d = {}
        for l in range(4):
            d[f'rbT_{l}'] = np.ascontiguousarray(
                (rb[l][eidx] * valid[:, None]).T).astype(bf)
        d['s_mat'] = (sph_cat[eidx] * valid[:, None]).astype(np.float32)
        d['nbr'] = np.ascontiguousarray(
            (neighbors[eidx] * valid.astype(np.int64)).astype(np.int32)
            [:, None])
        d['ind'] = ind.reshape(NT * 128, 128).astype(bf)
        a0c, a1c = abnd[c], abnd[c + 1]
        for l in range(4):
            fo = np.zeros((NCH * 128, (2 * l + 1) * KMAX[l]), np.float32)
            fo[:a1c - a0c] = feats[l][a0c:a1c].reshape(a1c - a0c, -1)
            d[f'featown_{l}'] = fo
        per_core.append(d)

    rep = {}
    for l in (1, 2, 3):
        B, m = TBB[l], MSZ[l]
        cf = _cfeat(feats, l)
        ng = (KW + B - 1) // B
        t = np.zeros((B * m, ng, N_ATOMS), np.float32)
        for g in range(ng):
            for kb in range(B):
                k = g * B + kb
                if k < KW:
                    t[kb * m:(kb + 1) * m, g, :] = cf[:, :, k].T
        rep[f'cfT_{l}'] = t.astype(bf)
        uf = _uflat(U[PADDED_L[l]])[:m, :]
        rep[f'Utab_{l}'] = _blockdiag(uf, B).astype(bf)
        rep[f'UVbd_{l}'] = _blockdiag(uf, VKB[l]).astype(bf)
    rep['g0tab'] = np.ascontiguousarray(feats[0][:, 0, 96:128]).astype(bf)
    wu = _build_wu(U, Wl)
    for l in range(4):
        rep[f'WU_{l}'] = wu[l].astype(bf)
        rep[f'Wr1_{l}'] = Wr1[l].astype(bf)
        rep[f'Wr2_{l}'] = Wr2[l].astype(bf)

    meta = dict(NT=NT, NCH=NCH, nt_s=nt_s, tbase=[int(x) for x in tbase],
                abnd=abnd, EP=EP)
    return per_core, rep, meta


def build_program(meta):
    import os
    PH = int(os.environ.get("KPHASE", "4"))
    NT, NCH, EP = meta['NT'], meta['NCH'], meta['EP']
    nt_s, tbase = meta['nt_s'], meta['tbase']

    nc = bacc.Bacc("TRN2", target_bir_lowering=False, debug=False,
                   num_devices=NC_)
    ctx = ExitStack()

    din = {}

    def dri(name, shape, dt):
        din[name] = nc.dram_tensor(name, shape, dt, kind="ExternalInput")

    for l in range(4):
        dri(f'rbT_{l}', [NMAX[l], EP], BF16)
        dri(f'featown_{l}', [NCH * 128, (2 * l + 1) * KMAX[l]], F32)
        dri(f'WU_{l}', [PTOT, (2 * l + 1) * KMAX[l]], BF16)
        dri(f'Wr1_{l}', [NMAX[l], HIDDEN], BF16)
        dri(f'Wr2_{l}', [HIDDEN, KMAX[l]], BF16)
    dri('s_mat', [EP, 16], F32)
    dri('nbr', [EP, 1], I32)
    dri('ind', [NT * 128, 128], BF16)
    for l in (1, 2, 3):
        B, m = TBB[l], MSZ[l]
        ng = (KW + B - 1) // B
        dri(f'cfT_{l}', [B * m, ng, N_ATOMS], BF16)
        dri(f'Utab_{l}', [B * m, B * SIDE[l] ** 2], BF16)
        dri(f'UVbd_{l}', [VKB[l] * m, VKB[l] * ITSZ[l]], BF16)
    dri('g0tab', [N_ATOMS, KW], BF16)
    douts = [nc.dram_tensor(f'out_{l}', [NCH * 128, (2 * l + 1) * KMAX[l]],
                            F32, kind="ExternalOutput") for l in range(4)]
    gtab = nc.dram_tensor('gtab', [N_ATOMS, GROW], BF16)

    with TileContext(nc) as tc:
        cpool = ctx.enter_context(tc.tile_pool(name="const", bufs=1))
        ident = cpool.tile([128, 128], BF16)
        make_identity(nc, ident[:])

        sb = {}
        for name in ('Utab_1', 'Utab_2', 'Utab_3', 'UVbd_1', 'UVbd_2',
                     'UVbd_3', 'Wr1_0', 'Wr1_1', 'Wr1_2', 'Wr1_3',
                     'Wr2_0', 'Wr2_1', 'Wr2_2', 'Wr2_3'):
            t = din[name]
            sb[name] = cpool.tile(list(t.shape), t.dtype, name=name)
            nc.sync.dma_start(sb[name][:], t.ap())
        for l in range(4):
            t = din[f'WU_{l}']
            w = cpool.tile([128, NPIECE, t.shape[1]], BF16, name=f'wu{l}')
            sb[f'WU_{l}'] = w
            nc.sync.dma_start(w[:],
                              t.ap().rearrange("(p q) c -> q p c", q=128))
        s_sb = cpool.tile([128, NT, 16], F32)
        nc.sync.dma_start(
            s_sb[:], din['s_mat'].ap().rearrange("(t q) m -> q t m", q=128))
        nbr_all = cpool.tile([128, NT], I32)
        nc.sync.dma_start(
            nbr_all[:], din['nbr'].ap().rearrange("(t q) one -> q (t one)",
                                                  q=128))
        ind_all = cpool.tile([128, NT, 128], BF16)
        nc.sync.dma_start(
            ind_all[:], din['ind'].ap().rearrange("(t q) a -> q t a", q=128))
        rbT_sb = {}
        for l in range(4):
            rbT_sb[l] = cpool.tile([NMAX[l], EP], BF16, name=f'rbt{l}')
            nc.sync.dma_start(rbT_sb[l][:], din[f'rbT_{l}'].ap())

        # -------- phase 1: radial MLP --------
        radial_sb = cpool.tile([128, NT, 320], F32)
        if PH >= 1:
         with tc.tile_pool(name="p1ps", bufs=2, space="PSUM") as pp1, \
                tc.tile_pool(name="p1sb", bufs=2) as ps1:
            for l in range(4):
                for t0 in range(0, NT, 4):
                    nt4 = min(4, NT - t0)
                    ec = nt4 * 128
                    h_ps = pp1.tile([HIDDEN, 512], F32, tag="h")
                    nc.tensor.matmul(h_ps[:, :ec], lhsT=sb[f'Wr1_{l}'][:],
                                     rhs=rbT_sb[l][:, t0 * 128:t0 * 128 + ec],
                                     start=True, stop=True)
                    h_sg = ps1.tile([HIDDEN, 512], F32, tag="hsg")
                    nc.scalar.activation(h_sg[:, :ec], h_ps[:, :ec],
                                         mybir.ActivationFunctionType.Sigmoid)
                    h_sb = ps1.tile([HIDDEN, 512], BF16, tag="hs")
                    nc.vector.tensor_tensor(out=h_sb[:, :ec],
                                            in0=h_ps[:, :ec],
                                            in1=h_sg[:, :ec],
                                            op=mybir.AluOpType.mult)
                    for ti in range(nt4):
                        r_ps = pp1.tile([128, KMAX[l]], F32, tag="r")
                        nc.tensor.matmul(r_ps[:],
                                         lhsT=h_sb[:, ti * 128:ti * 128 + 128],
                                         rhs=sb[f'Wr2_{l}'][:],
                                         start=True, stop=True)
                        nc.scalar.copy(
                            radial_sb[:, t0 + ti,
                                      KOFF[l]:KOFF[l] + KMAX[l]], r_ps[:])

        # -------- phase 2: G-table build --------
        if PH >= 2:
         with tc.tile_pool(name="p2ps", bufs=2, space="PSUM") as pp2, \
                tc.tile_pool(name="p2sb", bufs=2) as ps2:
            for a0 in range(0, N_ATOMS, 512):
                ac = min(512, N_ATOMS - a0)
                nq = (ac + 127) // 128
                grows = ps2.tile([128, 4, GOFF[0]], BF16, tag="grows")
                for l in (3, 2, 1):
                    B, m, s2 = TBB[l], MSZ[l], SIDE[l] ** 2
                    ng = (KW + B - 1) // B
                    cf = ps2.tile([B * m, ng, 512], BF16, tag=f"cf{l}")
                    nc.sync.dma_start(cf[:, :, :ac],
                                      din[f'cfT_{l}'].ap()[:, :, a0:a0 + ac])
                    for g in range(ng):
                        nkb = min(B, KW - g * B)
                        gt_ps = pp2.tile([B * s2, 512], F32, tag="gt")
                        nc.tensor.matmul(gt_ps[:, :ac],
                                         lhsT=sb[f'Utab_{l}'][:],
                                         rhs=cf[:, g, :ac],
                                         start=True, stop=True)
                        gt_sb = ps2.tile([B * s2, 512], BF16, tag="gts")
                        nc.scalar.copy(gt_sb[:, :ac], gt_ps[:, :ac])
                        for q in range(nq):
                            an = min(128, ac - q * 128)
                            tp = pp2.tile([128, B * s2], BF16, tag="tp")
                            nc.tensor.transpose(
                                tp[:an, :], gt_sb[:, q * 128:q * 128 + an],
                                ident[:B * s2, :B * s2])
                            dst = grows[:an, q, :].rearrange(
                                "p (tj k) -> p tj k", k=KW)[
                                :, GOFF[l] // KW:GOFF[l] // KW + s2,
                                g * B:g * B + nkb]
                            src = tp[:an, :].rearrange(
                                "p (kb tj) -> p kb tj", kb=B)[
                                :, :nkb, :].rearrange("p kb tj -> p tj kb")
                            nc.vector.tensor_copy(dst, src)
                g0s = ps2.tile([128, 4, KW], BF16, tag="g0s")
                for q in range(nq):
                    an = min(128, ac - q * 128)
                    nc.sync.dma_start(
                        g0s[:an, q, :],
                        din['g0tab'].ap()[a0 + q * 128:a0 + q * 128 + an, :])
                    nc.sync.dma_start(
                        gtab.ap()[a0 + q * 128:a0 + q * 128 + an, :GOFF[0]],
                        grows[:an, q, :])
                    nc.sync.dma_start(
                        gtab.ap()[a0 + q * 128:a0 + q * 128 + an,
                                  GOFF[0]:GOFF[0] + KW],
                        g0s[:an, q, :])

        # -------- phase 3: edge loop --------
        coff = {3: 0, 2: 512, 1: 800, 0: 928}
        if PH >= 3:
         with tc.tile_pool(name="plps", bufs=1, space="PSUM") as poolp, \
                tc.tile_pool(name="p3ps", bufs=1, space="PSUM") as pp3, \
                tc.tile_pool(name="p3sb", bufs=2) as ps3, \
                tc.tile_pool(name="pout", bufs=1) as pout:
            for s_i in range(NCH):
                np3 = [poolp.tile([128, 400], F32, tag="pl3a", name="pl3a"),
                       poolp.tile([128, 400], F32, tag="pl3b", name="pl3b")]
                np2 = poolp.tile([128, 288], F32, tag="pl2", name="pl2")
                np10 = poolp.tile([128, 320], F32, tag="pl10", name="pl10")
                ntl = min(nt_s[s_i], int(os.environ.get("KTILES", "999")))
                for tloc in range(ntl):
                    ti = tbase[s_i] + tloc
                    first = tloc == 0
                    last = tloc == ntl - 1
                    # cvec
                    cvec = ps3.tile([128, 960], BF16, tag="cvec")
                    for l in range(4):
                        for lp in range(l + 1):
                            mlo, msz = lp * lp, 2 * lp + 1
                            dst = cvec[:, coff[l]:coff[l] + KW * MSZ[l]]\
                                .rearrange("p (k m) -> p k m", k=KW)\
                                [:, :, mlo:mlo + msz]
                            s_in = s_sb[:, ti, mlo:mlo + msz].unsqueeze(1)\
                                .broadcast_to([128, KW, msz])
                            r_in = radial_sb[:, ti, KOFF[lp] + LO[l]:
                                             KOFF[lp] + LO[l] + KW]\
                                .unsqueeze(2).broadcast_to([128, KW, msz])
                            nc.vector.tensor_tensor(out=dst, in0=s_in,
                                                    in1=r_in,
                                                    op=mybir.AluOpType.mult)
                    # transposes
                    KS = int(os.environ.get("KSTAGE", "9"))
                    cvT = {3: [], 2: [], 1: []}
                    blocks = [(3, 0, 128), (3, 128, 128), (3, 256, 128),
                              (3, 384, 128),
                              (2, 512, 72), (2, 584, 72),
                              (2, 656, 72), (2, 728, 72),
                              (1, 800, 128)]
                    for bi, (l, off, w) in enumerate(blocks) if KS >= 2 else []:
                        tp = pp3.tile([128, 128], BF16, tag="tp", bufs=2)
                        nc.tensor.transpose(tp[:w, :], cvec[:, off:off + w],
                                            ident[:])
                        piece = ps3.tile([128, 128], BF16, tag=f"cvT{bi}", name=f"cvT{bi}")
                        nc.scalar.copy(piece[:w, :], tp[:w, :])
                        cvT[l].append(piece)
                    # uncouple V -> vsb (it,k) layout bf16
                    vsb3 = ps3.tile([128, 800], BF16, tag="vs3", name="vs3")
                    vsb = {2: ps3.tile([128, 288], BF16, tag="vs2", name="vs2"),
                           1: ps3.tile([128, 288], BF16, tag="vs1", name="vs1")}
                    _lset = tuple(int(x) for x in os.environ.get(
                        "KLSET", "321"))
                    for l in (_lset if KS >= 3 else []):
                        nkb, m, it = VKB[l], MSZ[l], ITSZ[l]
                        if l == 3:
                            for h in range(2):
                                vp = pp3.tile([128, 400], F32, tag="vv", bufs=2,
                                              name="vv")
                                for qq in range(2):
                                    q = h * 2 + qq
                                    nc.tensor.matmul(
                                        vp[:, qq * 200:qq * 200 + 200],
                                        lhsT=cvT[3][q][:128, :],
                                        rhs=sb['UVbd_3'][:],
                                        start=True, stop=True)
                                if "KNOCOPY" not in os.environ:
                                    srcap = vp[:].rearrange(
                                        "p (kq kk it) -> p kq kk it",
                                        kq=2, kk=8)\
                                        .rearrange("p kq kk it -> p kq it kk")
                                    dst = vsb3[:].rearrange(
                                        "p (it k) -> p it k", k=KW)\
                                        [:, :, h * 16:h * 16 + 16].rearrange(
                                        "p it (kq kk) -> p kq it kk", kq=2)
                                    nc.scalar.copy(dst, srcap)
                        elif l == 2:
                            vp = pp3.tile([128, 288], F32, tag="vv", bufs=2, name="vv")
                            for q in range(4):
                                nc.tensor.matmul(
                                    vp[:, q * 72:q * 72 + 72],
                                    lhsT=cvT[2][q][:72, :],
                                    rhs=sb['UVbd_2'][:],
                                    start=True, stop=True)
                            if "KNOCOPY" not in os.environ:
                                srcap = vp[:].rearrange(
                                    "p (kq kk it) -> p kq kk it", kq=4, kk=8)\
                                    .rearrange("p kq kk it -> p kq it kk")
                                dst = vsb[2][:].rearrange(
                                    "p (it kq kk) -> p kq it kk", kq=4, kk=8)
                                nc.scalar.copy(dst, srcap)
                        else:
                            vp = pp3.tile([128, 288], F32, tag="vv", bufs=2, name="vv")
                            nc.tensor.matmul(vp[:], lhsT=cvT[1][0][:128, :],
                                             rhs=sb['UVbd_1'][:],
                                             start=True, stop=True)
                            if "KNOCOPY" not in os.environ:
                                srcap = vp[:].rearrange(
                                    "p (kk it) -> p kk it", kk=32)\
                                    .rearrange("p kk it -> p it kk")
                                dst = vsb[1][:].rearrange(
                                    "p (it kk) -> p it kk", kk=32)
                                nc.scalar.copy(dst, srcap)
                    # gather
                    G = ps3.tile([128, GROW], BF16, tag="G")
                    if PH >= 4:
                        nc.gpsimd.indirect_dma_start(
                            out=G[:], out_offset=None, in_=gtab.ap()[:, :],
                            in_offset=IndirectOffsetOnAxis(
                                ap=nbr_all[:, ti:ti + 1], axis=0))
                    else:
                        nc.gpsimd.memset(G[:], 0.0)
                    # products
                    PT = [ps3.tile([128, 1408], BF16, tag=f"PT{t}", name=f"PT{t}")
                          for t in range(5)]
                    for t in range(5) if KS >= 4 else []:
                        v = vsb3[:].rearrange(
                            "p (i t k) -> p i t k", i=5, k=KW)\
                            [:, :, t, :].unsqueeze(2)\
                            .broadcast_to([128, 5, 5, KW])
                        g3 = G[:, GOFF[3]:GOFF[3] + 800].rearrange(
                            "p (t j k) -> p t j k", t=5, k=KW)\
                            [:, t, :, :].unsqueeze(1)\
                            .broadcast_to([128, 5, 5, KW])
                        o = PT[t][:, 0:800].rearrange(
                            "p (i j k) -> p i j k", i=5, k=KW)
                        nc.vector.tensor_tensor(out=o, in0=v, in1=g3,
                                                op=mybir.AluOpType.mult)
                    for l in (2, 1) if KS >= 4 else []:
                        for t in range(3):
                            v = vsb[l][:].rearrange(
                                "p (i t k) -> p i t k", i=3, k=KW)\
                                [:, :, t, :].unsqueeze(2)\
                                .broadcast_to([128, 3, 3, KW])
                            gl = G[:, GOFF[l]:GOFF[l] + 288].rearrange(
                                "p (t j k) -> p t j k", t=3, k=KW)\
                                [:, t, :, :].unsqueeze(1)\
                                .broadcast_to([128, 3, 3, KW])
                            o = PT[t][:, GOFF[l]:GOFF[l] + 288]\
                                .rearrange("p (i j k) -> p i j k", i=3, k=KW)
                            nc.vector.tensor_tensor(out=o, in0=v, in1=gl,
                                                    op=mybir.AluOpType.mult)
                    if KS >= 4:
                     nc.vector.tensor_tensor(
                        out=PT[0][:, 1376:1408], in0=cvec[:, 928:960],
                        in1=G[:, GOFF[0]:GOFF[0] + KW],
                        op=mybir.AluOpType.mult)
                    # segment matmuls
                    if KS < 5:
                        continue
                    lhs_ind = ind_all[:, ti, :]
                    for t in range(5):
                        for h in range(2):
                            nc.tensor.matmul(
                                np3[h][:], lhsT=lhs_ind,
                                rhs=PT[t][:, h * 400:h * 400 + 400],
                                start=(first and t == 0),
                                stop=(last and t == 4))
                    for t in range(3):
                        nc.tensor.matmul(np2[:], lhsT=lhs_ind,
                                         rhs=PT[t][:, 800:1088],
                                         start=(first and t == 0),
                                         stop=(last and t == 2))
                    nc.tensor.matmul(np10[:], lhsT=lhs_ind,
                                     rhs=PT[0][:, 1088:1408],
                                     start=first, stop=False)
                    for t in (1, 2):
                        nc.tensor.matmul(np10[:, :288], lhsT=lhs_ind,
                                         rhs=PT[t][:, 1088:1376],
                                         start=False, stop=(last and t == 2))
                # ---- chunk epilogue ----
                if int(os.environ.get("KSTAGE", "9")) < 5:
                    continue
                pooled = pout.tile([128, PTOT], BF16, tag="pooled")
                nc.scalar.copy(pooled[:, 0:400], np3[0][:])
                nc.scalar.copy(pooled[:, 400:800], np3[1][:])
                nc.scalar.copy(pooled[:, 800:1088], np2[:])
                nc.scalar.copy(pooled[:, 1088:1408], np10[:])
                pieces = pout.tile([128, NPIECE, 128], BF16, tag="pieces")
                for p in range(NPIECE):
                    tp = pp3.tile([128, 128], BF16, tag="tp", bufs=2)
                    nc.tensor.transpose(tp[:], pooled[:, p * 128:p * 128 + 128],
                                        ident[:])
                    nc.scalar.copy(pieces[:, p, :], tp[:])
                for l in range(4):
                    ncol = (2 * l + 1) * KMAX[l]
                    fo = pout.tile([128, 960], F32, tag="fo")
                    nc.sync.dma_start(
                        fo[:, :ncol],
                        din[f'featown_{l}'].ap()[s_i * 128:s_i * 128 + 128, :])
                    for c0 in range(0, ncol, 128):
                        cw = min(128, ncol - c0)
                        ops_t = pp3.tile([128, 400], F32, tag="vv", bufs=2,
                                         name="ops")
                        ops = ops_t[:, 0:128]
                        for p in range(NPIECE):
                            nc.tensor.matmul(
                                ops[:cw, :],
                                lhsT=sb[f'WU_{l}'][:, p, c0:c0 + cw],
                                rhs=pieces[:, p, :],
                                start=(p == 0), stop=(p == NPIECE - 1))
                        osb = pout.tile([128, 128], BF16, tag="osb")
                        nc.scalar.copy(osb[:cw, :], ops[:cw, :])
                        tp2 = pp3.tile([128, 128], BF16, tag="tp", bufs=2)
                        nc.tensor.transpose(tp2[:, :cw], osb[:cw, :],
                                            ident[:cw, :cw])
                        ofin = pout.tile([128, 128], F32, tag="ofin", bufs=2)
                        nc.vector.tensor_add(out=ofin[:, :cw],
                                             in0=tp2[:, :cw],
                                             in1=fo[:, c0:c0 + cw])
                        nc.sync.dma_start(
                            douts[l].ap()[s_i * 128:s_i * 128 + 128,
                                          c0:c0 + cw], ofin[:, :cw])
        ctx.close()
    nc.compile()
    return nc


def kernel(**inputs):
    per_core, rep, meta = _host_prep(inputs)
    nc = build_program(meta)
    in_maps = []
    for c in range(NC_):
        m = dict(per_core[c])
        m.update(rep)
        in_maps.append(m)
    res = run_bass_kernel_spmd(nc, in_maps, list(range(NC_)))
    outs = []
    abnd = meta['abnd']
    for l in range(4):
        full = np.zeros((N_ATOMS, 2 * l + 1, KMAX[l]), np.float32)
        for c in range(NC_):
            a0, a1 = abnd[c], abnd[c + 1]
            full[a0:a1] = res.results[c][f'out_{l}'][:a1 - a0].reshape(
                a1 - a0, 2 * l + 1, KMAX[l])
        outs.append(full)
    return tuple(outs)


# revision 24
# speedup vs baseline: 1.2368x; 1.0235x over previous
"""Trainium2 Bass kernel for nn_EquivariantMessagePasser (8-core SPMD).

Strategy: edges sorted+sharded by center atom (segment-sum is core-local via
per-tile indicator matmuls into accumulating PSUM); feats replicated; per-atom
uncoupled-feature table built on device in DRAM and gathered per edge via
indirect DMA; couple-back + output linear folded into one host-precomputed
weight (WU = U x Wl).
"""
import sys

sys.path.insert(0, "/opt/trn_rl_repo")

from contextlib import ExitStack

import numpy as np
import ml_dtypes

import concourse.bass as bass
from concourse import bacc, mybir
from concourse.bass import IndirectOffsetOnAxis
from concourse.tile import TileContext
from concourse.bass_utils import run_bass_kernel_spmd
from concourse.masks import make_identity

F32 = mybir.dt.float32
BF16 = mybir.dt.bfloat16
I32 = mybir.dt.int32
bf = ml_dtypes.bfloat16

NMAX = [8, 6, 4, 2]
KMAX = [128, 96, 64, 32]
PADDED_L = [0, 2, 2, 4]
SIDE = [1, 3, 3, 5]
MSZ = [1, 4, 9, 16]
ITSZ = [1, 9, 9, 25]
N_ATOMS = 2500
HIDDEN = 64
NC_ = 8
KW = 32
LO = [96, 64, 32, 0]
KOFF = [0, 128, 224, 288]         # radial region offsets (cumsum KMAX)

# row layout: [l3 (i/t,j,k)=800 | l2 288 | l1 288 | l0 32] = 1408
GOFF = {3: 0, 2: 800, 1: 1088, 0: 1376}
GROW = 1408
PTOT = 1408
NPIECE = 11

VKB = {3: 8, 2: 8, 1: 32}         # k-channels per uncouple-V matmul
TBB = {3: 5, 2: 14, 1: 14}        # k-channels per table-build block


def _uflat(U):
    side = U.shape[0]
    return U.reshape(side * side, side * side).T.copy()  # [m, (i,j)]


def _blockdiag(mat, B):
    m, n = mat.shape
    out = np.zeros((B * m, B * n), mat.dtype)
    for b in range(B):
        out[b * m:(b + 1) * m, b * n:(b + 1) * n] = mat
    return out


def _cfeat(feats, l):
    return np.concatenate(
        [feats[lp][:, :, LO[l]:LO[l] + KW] for lp in range(l + 1)], axis=1)


def _build_wu(U, Wl):
    u0 = float(np.asarray(U[0]).reshape(-1)[0])
    wu = []
    for l in range(4):
        ncol = (2 * l + 1) * KMAX[l]
        M = np.zeros((PTOT, ncol), np.float32)
        for lch in range(l, 4):
            s = SIDE[lch]
            uf = np.asarray(U[PADDED_L[lch]], np.float32)
            koff = (lch - l) * KW
            for i in range(s):
                for j in range(s):
                    for mloc in range(2 * l + 1):
                        uv = float(uf[i, j, l * l + mloc])
                        if lch == 0:
                            uv *= u0 * u0
                        r0 = GOFF[lch] + (i * s + j) * KW
                        M[r0:r0 + KW, mloc * KMAX[l]:(mloc + 1) * KMAX[l]] += \
                            uv * Wl[l][koff:koff + KW, :]
        wu.append(M)
    return wu


def _host_prep(inp):
    rb = [np.asarray(inp[f'radial_basis_{l}'], np.float32) for l in range(4)]
    sph = [np.asarray(inp[f'spherical_harmonics_{l}'], np.float32)
           for l in range(4)]
    feats = [np.asarray(inp[f'features_{l}'], np.float32) for l in range(4)]
    centers = np.asarray(inp['centers'])
    neighbors = np.asarray(inp['neighbors'])
    U = {L: np.asarray(inp[f'U{L}'], np.float32) for L in (0, 2, 4)}
    Wr1 = [np.asarray(inp[f'Wr1_{l}'], np.float32) for l in range(4)]
    Wr2 = [np.asarray(inp[f'Wr2_{l}'], np.float32) for l in range(4)]
    Wl = [np.asarray(inp[f'Wl_{l}'], np.float32) for l in range(4)]

    order = np.argsort(centers, kind='stable')
    c_sorted = centers[order]
    abnd = [round(c * N_ATOMS / NC_) for c in range(NC_ + 1)]
    starts = np.searchsorted(c_sorted, np.arange(N_ATOMS + 1))
    core_chunks = []
    nch_max = 0
    for c in range(NC_):
        a0c, a1c = abnd[c], abnd[c + 1]
        chunks = []
        a = a0c
        while a < a1c:
            na = min(128, a1c - a)
            e0, e1 = int(starts[a]), int(starts[a + na])
            chunks.append((a, na, e0, e1 - e0))
            a += na
        core_chunks.append(chunks)
        nch_max = max(nch_max, len(chunks))
    NCH = nch_max
    for c in range(NC_):
        while len(core_chunks[c]) < NCH:
            core_chunks[c].append((abnd[c + 1], 0, 0, 0))
    nt_s = [max((core_chunks[c][s][3] + 127) // 128 for c in range(NC_))
            for s in range(NCH)]
    NT = sum(nt_s)
    EP = NT * 128
    tbase = np.cumsum([0] + nt_s)

    sph_cat = np.concatenate(sph, axis=1)

    per_core = []
    for c in range(NC_):
        eidx = np.zeros(EP, np.int64)
        valid = np.zeros(EP, np.float32)
        ind = np.zeros((NT, 128, 128), np.float32)
        for s_i, (a0, na, e0, ne) in enumerate(core_chunks[c]):
            pos0 = int(tbase[s_i]) * 128
            idx = order[e0:e0 + ne]
            eidx[pos0:pos0 + ne] = idx
            valid[pos0:pos0 + ne] = 1.0
            loc = c_sorted[e0:e0 + ne] - a0
            rows = np.arange(pos0, pos0 + ne)
            ind[rows // 128, rows % 128, loc] = 1.0
        d = {}
        for l in range(4):
            d[f'rbT_{l}'] = np.ascontiguousarray(
                (rb[l][eidx] * valid[:, None]).T).astype(bf)
        d['s_mat'] = (sph_cat[eidx] * valid[:, None]).astype(np.float32)
        d['nbr'] = np.ascontiguousarray(
            (neighbors[eidx] * valid.astype(np.int64)).astype(np.int32)
            [:, None])
        d['ind'] = ind.reshape(NT * 128, 128).astype(bf)
        a0c, a1c = abnd[c], abnd[c + 1]
        for l in range(4):
            fo = np.zeros((NCH * 128, (2 * l + 1) * KMAX[l]), np.float32)
            fo[:a1c - a0c] = feats[l][a0c:a1c].reshape(a1c - a0c, -1)
            d[f'featown_{l}'] = fo
        per_core.append(d)

    rep = {}
    for l in (1, 2, 3):
        B, m = TBB[l], MSZ[l]
        cf = _cfeat(feats, l)
        ng = (KW + B - 1) // B
        t = np.zeros((B * m, ng, N_ATOMS), np.float32)
        for g in range(ng):
            for kb in range(B):
                k = g * B + kb
                if k < KW:
                    t[kb * m:(kb + 1) * m, g, :] = cf[:, :, k].T
        rep[f'cfT_{l}'] = t.astype(bf)
        uf = _uflat(U[PADDED_L[l]])[:m, :]
        rep[f'Utab_{l}'] = _blockdiag(uf, B).astype(bf)
        rep[f'UVbd_{l}'] = _blockdiag(uf, VKB[l]).astype(bf)
    rep['g0tab'] = np.ascontiguousarray(feats[0][:, 0, 96:128]).astype(bf)
    wu = _build_wu(U, Wl)
    for l in range(4):
        rep[f'WU_{l}'] = wu[l].astype(bf)
        rep[f'Wr1_{l}'] = Wr1[l].astype(bf)
        rep[f'Wr2_{l}'] = Wr2[l].astype(bf)

    meta = dict(NT=NT, NCH=NCH, nt_s=nt_s, tbase=[int(x) for x in tbase],
                abnd=abnd, EP=EP)
    return per_core, rep, meta


def build_program(meta):
    import os
    PH = int(os.environ.get("KPHASE", "4"))
    NT, NCH, EP = meta['NT'], meta['NCH'], meta['EP']
    nt_s, tbase = meta['nt_s'], meta['tbase']

    nc = bacc.Bacc("TRN2", target_bir_lowering=False, debug=False,
                   num_devices=NC_)
    ctx = ExitStack()

    din = {}

    def dri(name, shape, dt):
        din[name] = nc.dram_tensor(name, shape, dt, kind="ExternalInput")

    for l in range(4):
        dri(f'rbT_{l}', [NMAX[l], EP], BF16)
        dri(f'featown_{l}', [NCH * 128, (2 * l + 1) * KMAX[l]], F32)
        dri(f'WU_{l}', [PTOT, (2 * l + 1) * KMAX[l]], BF16)
        dri(f'Wr1_{l}', [NMAX[l], HIDDEN], BF16)
        dri(f'Wr2_{l}', [HIDDEN, KMAX[l]], BF16)
    dri('s_mat', [EP, 16], F32)
    dri('nbr', [EP, 1], I32)
    dri('ind', [NT * 128, 128], BF16)
    for l in (1, 2, 3):
        B, m = TBB[l], MSZ[l]
        ng = (KW + B - 1) // B
        dri(f'cfT_{l}', [B * m, ng, N_ATOMS], BF16)
        dri(f'Utab_{l}', [B * m, B * SIDE[l] ** 2], BF16)
        dri(f'UVbd_{l}', [VKB[l] * m, VKB[l] * ITSZ[l]], BF16)
    dri('g0tab', [N_ATOMS, KW], BF16)
    douts = [nc.dram_tensor(f'out_{l}', [NCH * 128, (2 * l + 1) * KMAX[l]],
                            F32, kind="ExternalOutput") for l in range(4)]
    gtab = nc.dram_tensor('gtab', [N_ATOMS, GROW], BF16)

    with TileContext(nc) as tc:
        cpool = ctx.enter_context(tc.tile_pool(name="const", bufs=1))
        ident = cpool.tile([128, 128], BF16)
        make_identity(nc, ident[:])

        sb = {}
        for name in ('Utab_1', 'Utab_2', 'Utab_3', 'UVbd_1', 'UVbd_2',
                     'UVbd_3', 'Wr1_0', 'Wr1_1', 'Wr1_2', 'Wr1_3',
                     'Wr2_0', 'Wr2_1', 'Wr2_2', 'Wr2_3'):
            t = din[name]
            sb[name] = cpool.tile(list(t.shape), t.dtype, name=name)
            nc.sync.dma_start(sb[name][:], t.ap())
        for l in range(4):
            t = din[f'WU_{l}']
            w = cpool.tile([128, NPIECE, t.shape[1]], BF16, name=f'wu{l}')
            sb[f'WU_{l}'] = w
            nc.sync.dma_start(w[:],
                              t.ap().rearrange("(p q) c -> q p c", q=128))
        s_sb = cpool.tile([128, NT, 16], F32)
        nc.sync.dma_start(
            s_sb[:], din['s_mat'].ap().rearrange("(t q) m -> q t m", q=128))
        nbr_all = cpool.tile([128, NT], I32)
        nc.sync.dma_start(
            nbr_all[:], din['nbr'].ap().rearrange("(t q) one -> q (t one)",
                                                  q=128))
        ind_all = cpool.tile([128, NT, 128], BF16)
        nc.sync.dma_start(
            ind_all[:], din['ind'].ap().rearrange("(t q) a -> q t a", q=128))
        rbT_sb = {}
        for l in range(4):
            rbT_sb[l] = cpool.tile([NMAX[l], EP], BF16, name=f'rbt{l}')
            nc.sync.dma_start(rbT_sb[l][:], din[f'rbT_{l}'].ap())

        # -------- phase 1: radial MLP --------
        radial_sb = cpool.tile([128, NT, 320], F32)
        if PH >= 1:
         with tc.tile_pool(name="p1ps", bufs=2, space="PSUM") as pp1, \
                tc.tile_pool(name="p1sb", bufs=2) as ps1:
            for l in range(4):
                for t0 in range(0, NT, 4):
                    nt4 = min(4, NT - t0)
                    ec = nt4 * 128
                    h_ps = pp1.tile([HIDDEN, 512], F32, tag="h")
                    nc.tensor.matmul(h_ps[:, :ec], lhsT=sb[f'Wr1_{l}'][:],
                                     rhs=rbT_sb[l][:, t0 * 128:t0 * 128 + ec],
                                     start=True, stop=True)
                    h_sg = ps1.tile([HIDDEN, 512], F32, tag="hsg")
                    nc.scalar.activation(h_sg[:, :ec], h_ps[:, :ec],
                                         mybir.ActivationFunctionType.Sigmoid)
                    h_sb = ps1.tile([HIDDEN, 512], BF16, tag="hs")
                    nc.vector.tensor_tensor(out=h_sb[:, :ec],
                                            in0=h_ps[:, :ec],
                                            in1=h_sg[:, :ec],
                                            op=mybir.AluOpType.mult)
                    for ti in range(nt4):
                        r_ps = pp1.tile([128, KMAX[l]], F32, tag="r")
                        nc.tensor.matmul(r_ps[:],
                                         lhsT=h_sb[:, ti * 128:ti * 128 + 128],
                                         rhs=sb[f'Wr2_{l}'][:],
                                         start=True, stop=True)
                        nc.scalar.copy(
                            radial_sb[:, t0 + ti,
                                      KOFF[l]:KOFF[l] + KMAX[l]], r_ps[:])

        # -------- phase 2: G-table build --------
        if PH >= 2:
         with tc.tile_pool(name="p2ps", bufs=2, space="PSUM") as pp2, \
                tc.tile_pool(name="p2sb", bufs=2) as ps2:
            for a0 in range(0, N_ATOMS, 512):
                ac = min(512, N_ATOMS - a0)
                nq = (ac + 127) // 128
                grows = ps2.tile([128, 4, GOFF[0]], BF16, tag="grows")
                for l in (3, 2, 1):
                    B, m, s2 = TBB[l], MSZ[l], SIDE[l] ** 2
                    ng = (KW + B - 1) // B
                    cf = ps2.tile([B * m, ng, 512], BF16, tag=f"cf{l}")
                    nc.sync.dma_start(cf[:, :, :ac],
                                      din[f'cfT_{l}'].ap()[:, :, a0:a0 + ac])
                    for g in range(ng):
                        nkb = min(B, KW - g * B)
                        gt_ps = pp2.tile([B * s2, 512], F32, tag="gt")
                        nc.tensor.matmul(gt_ps[:, :ac],
                                         lhsT=sb[f'Utab_{l}'][:],
                                         rhs=cf[:, g, :ac],
                                         start=True, stop=True)
                        gt_sb = ps2.tile([B * s2, 512], BF16, tag="gts")
                        nc.scalar.copy(gt_sb[:, :ac], gt_ps[:, :ac])
                        for q in range(nq):
                            an = min(128, ac - q * 128)
                            tp = pp2.tile([128, B * s2], BF16, tag="tp")
                            nc.tensor.transpose(
                                tp[:an, :], gt_sb[:, q * 128:q * 128 + an],
                                ident[:B * s2, :B * s2])
                            dst = grows[:an, q, :].rearrange(
                                "p (tj k) -> p tj k", k=KW)[
                                :, GOFF[l] // KW:GOFF[l] // KW + s2,
                                g * B:g * B + nkb]
                            src = tp[:an, :].rearrange(
                                "p (kb tj) -> p kb tj", kb=B)[
                                :, :nkb, :].rearrange("p kb tj -> p tj kb")
                            nc.vector.tensor_copy(dst, src)
                g0s = ps2.tile([128, 4, KW], BF16, tag="g0s")
                for q in range(nq):
                    an = min(128, ac - q * 128)
                    nc.sync.dma_start(
                        g0s[:an, q, :],
                        din['g0tab'].ap()[a0 + q * 128:a0 + q * 128 + an, :])
                    nc.sync.dma_start(
                        gtab.ap()[a0 + q * 128:a0 + q * 128 + an, :GOFF[0]],
                        grows[:an, q, :])
                    nc.sync.dma_start(
                        gtab.ap()[a0 + q * 128:a0 + q * 128 + an,
                                  GOFF[0]:GOFF[0] + KW],
                        g0s[:an, q, :])

        # -------- phase 3: edge loop --------
        coff = {3: 0, 2: 512, 1: 800, 0: 928}
        if PH >= 3:
         with tc.tile_pool(name="plps", bufs=1, space="PSUM") as poolp, \
                tc.tile_pool(name="p3ps", bufs=1, space="PSUM") as pp3, \
                tc.tile_pool(name="p3sb", bufs=2) as ps3, \
                tc.tile_pool(name="pout", bufs=1) as pout:
            pieces = pout.tile([128, NPIECE, NCH * 128], BF16,
                               tag="pieces", name="pieces")
            for s_i in range(NCH):
                np3 = [poolp.tile([128, 400], F32, tag="pl3a", name="pl3a"),
                       poolp.tile([128, 400], F32, tag="pl3b", name="pl3b")]
                np2 = poolp.tile([128, 288], F32, tag="pl2", name="pl2")
                np10 = poolp.tile([128, 320], F32, tag="pl10", name="pl10")
                ntl = min(nt_s[s_i], int(os.environ.get("KTILES", "999")))
                for tloc in range(ntl):
                    ti = tbase[s_i] + tloc
                    first = tloc == 0
                    last = tloc == ntl - 1
                    # cvec
                    cvec = ps3.tile([128, 960], BF16, tag="cvec")
                    for l in range(4):
                        for lp in range(l + 1):
                            mlo, msz = lp * lp, 2 * lp + 1
                            dst = cvec[:, coff[l]:coff[l] + KW * MSZ[l]]\
                                .rearrange("p (k m) -> p k m", k=KW)\
                                [:, :, mlo:mlo + msz]
                            s_in = s_sb[:, ti, mlo:mlo + msz].unsqueeze(1)\
                                .broadcast_to([128, KW, msz])
                            r_in = radial_sb[:, ti, KOFF[lp] + LO[l]:
                                             KOFF[lp] + LO[l] + KW]\
                                .unsqueeze(2).broadcast_to([128, KW, msz])
                            nc.vector.tensor_tensor(out=dst, in0=s_in,
                                                    in1=r_in,
                                                    op=mybir.AluOpType.mult)
                    # transposes
                    KS = int(os.environ.get("KSTAGE", "9"))
                    cvT = {3: [], 2: [], 1: []}
                    blocks = [(3, 0, 128), (3, 128, 128), (3, 256, 128),
                              (3, 384, 128),
                              (2, 512, 72), (2, 584, 72),
                              (2, 656, 72), (2, 728, 72),
                              (1, 800, 128)]
                    for bi, (l, off, w) in enumerate(blocks) if KS >= 2 else []:
                        tp = pp3.tile([128, 128], BF16, tag="tp", bufs=2)
                        nc.tensor.transpose(tp[:w, :], cvec[:, off:off + w],
                                            ident[:])
                        piece = ps3.tile([128, 128], BF16, tag=f"cvT{bi}", name=f"cvT{bi}")
                        nc.scalar.copy(piece[:w, :], tp[:w, :])
                        cvT[l].append(piece)
                    # uncouple V -> vsb (it,k) layout bf16
                    vsb3 = ps3.tile([128, 800], BF16, tag="vs3", name="vs3")
                    vsb = {2: ps3.tile([128, 288], BF16, tag="vs2", name="vs2"),
                           1: ps3.tile([128, 288], BF16, tag="vs1", name="vs1")}
                    _lset = tuple(int(x) for x in os.environ.get(
                        "KLSET", "321"))
                    for l in (_lset if KS >= 3 else []):
                        nkb, m, it = VKB[l], MSZ[l], ITSZ[l]
                        if l == 3:
                            for h in range(2):
                                vp = pp3.tile([128, 400], F32, tag="vv", bufs=2,
                                              name="vv")
                                for qq in range(2):
                                    q = h * 2 + qq
                                    nc.tensor.matmul(
                                        vp[:, qq * 200:qq * 200 + 200],
                                        lhsT=cvT[3][q][:128, :],
                                        rhs=sb['UVbd_3'][:],
                                        start=True, stop=True)
                                if "KNOCOPY" not in os.environ:
                                    srcap = vp[:].rearrange(
                                        "p (kq kk it) -> p kq kk it",
                                        kq=2, kk=8)\
                                        .rearrange("p kq kk it -> p kq it kk")
                                    dst = vsb3[:].rearrange(
                                        "p (it k) -> p it k", k=KW)\
                                        [:, :, h * 16:h * 16 + 16].rearrange(
                                        "p it (kq kk) -> p kq it kk", kq=2)
                                    nc.scalar.copy(dst, srcap)
                        elif l == 2:
                            vp = pp3.tile([128, 288], F32, tag="vv", bufs=2, name="vv")
                            for q in range(4):
                                nc.tensor.matmul(
                                    vp[:, q * 72:q * 72 + 72],
                                    lhsT=cvT[2][q][:72, :],
                                    rhs=sb['UVbd_2'][:],
                                    start=True, stop=True)
                            if "KNOCOPY" not in os.environ:
                                srcap = vp[:].rearrange(
                                    "p (kq kk it) -> p kq kk it", kq=4, kk=8)\
                                    .rearrange("p kq kk it -> p kq it kk")
                                dst = vsb[2][:].rearrange(
                                    "p (it kq kk) -> p kq it kk", kq=4, kk=8)
                                nc.scalar.copy(dst, srcap)
                        else:
                            vp = pp3.tile([128, 288], F32, tag="vv", bufs=2, name="vv")
                            nc.tensor.matmul(vp[:], lhsT=cvT[1][0][:128, :],
                                             rhs=sb['UVbd_1'][:],
                                             start=True, stop=True)
                            if "KNOCOPY" not in os.environ:
                                srcap = vp[:].rearrange(
                                    "p (kk it) -> p kk it", kk=32)\
                                    .rearrange("p kk it -> p it kk")
                                dst = vsb[1][:].rearrange(
                                    "p (it kk) -> p it kk", kk=32)
                                nc.scalar.copy(dst, srcap)
                    # gather
                    G = ps3.tile([128, GROW], BF16, tag="G")
                    if PH >= 4:
                        nc.gpsimd.indirect_dma_start(
                            out=G[:], out_offset=None, in_=gtab.ap()[:, :],
                            in_offset=IndirectOffsetOnAxis(
                                ap=nbr_all[:, ti:ti + 1], axis=0))
                    else:
                        nc.gpsimd.memset(G[:], 0.0)
                    # products
                    PT = [ps3.tile([128, 1408], BF16, tag=f"PT{t}", name=f"PT{t}")
                          for t in range(5)]
                    for t in range(5) if KS >= 4 else []:
                        v = vsb3[:].rearrange(
                            "p (i t k) -> p i t k", i=5, k=KW)\
                            [:, :, t, :].unsqueeze(2)\
                            .broadcast_to([128, 5, 5, KW])
                        g3 = G[:, GOFF[3]:GOFF[3] + 800].rearrange(
                            "p (t j k) -> p t j k", t=5, k=KW)\
                            [:, t, :, :].unsqueeze(1)\
                            .broadcast_to([128, 5, 5, KW])
                        o = PT[t][:, 0:800].rearrange(
                            "p (i j k) -> p i j k", i=5, k=KW)
                        nc.vector.tensor_tensor(out=o, in0=v, in1=g3,
                                                op=mybir.AluOpType.mult)
                    for l in (2, 1) if KS >= 4 else []:
                        for t in range(3):
                            v = vsb[l][:].rearrange(
                                "p (i t k) -> p i t k", i=3, k=KW)\
                                [:, :, t, :].unsqueeze(2)\
                                .broadcast_to([128, 3, 3, KW])
                            gl = G[:, GOFF[l]:GOFF[l] + 288].rearrange(
                                "p (t j k) -> p t j k", t=3, k=KW)\
                                [:, t, :, :].unsqueeze(1)\
                                .broadcast_to([128, 3, 3, KW])
                            o = PT[t][:, GOFF[l]:GOFF[l] + 288]\
                                .rearrange("p (i j k) -> p i j k", i=3, k=KW)
                            nc.vector.tensor_tensor(out=o, in0=v, in1=gl,
                                                    op=mybir.AluOpType.mult)
                    if KS >= 4:
                     nc.vector.tensor_tensor(
                        out=PT[0][:, 1376:1408], in0=cvec[:, 928:960],
                        in1=G[:, GOFF[0]:GOFF[0] + KW],
                        op=mybir.AluOpType.mult)
                    # segment matmuls
                    if KS < 5:
                        continue
                    lhs_ind = ind_all[:, ti, :]
                    for t in range(5):
                        for h in range(2):
                            nc.tensor.matmul(
                                np3[h][:], lhsT=lhs_ind,
                                rhs=PT[t][:, h * 400:h * 400 + 400],
                                start=(first and t == 0),
                                stop=(last and t == 4))
                    for t in range(3):
                        nc.tensor.matmul(np2[:], lhsT=lhs_ind,
                                         rhs=PT[t][:, 800:1088],
                                         start=(first and t == 0),
                                         stop=(last and t == 2))
                    nc.tensor.matmul(np10[:], lhsT=lhs_ind,
                                     rhs=PT[0][:, 1088:1408],
                                     start=first, stop=False)
                    for t in (1, 2):
                        nc.tensor.matmul(np10[:, :288], lhsT=lhs_ind,
                                         rhs=PT[t][:, 1088:1376],
                                         start=False, stop=(last and t == 2))
                # ---- chunk epilogue ----
                if int(os.environ.get("KSTAGE", "9")) < 5:
                    continue
                pooled = pout.tile([128, PTOT], BF16, tag="pooled")
                nc.scalar.copy(pooled[:, 0:400], np3[0][:])
                nc.scalar.copy(pooled[:, 400:800], np3[1][:])
                nc.scalar.copy(pooled[:, 800:1088], np2[:])
                nc.scalar.copy(pooled[:, 1088:1408], np10[:])
                for p in range(NPIECE):
                    tp = pp3.tile([128, 128], BF16, tag="tp", bufs=2)
                    nc.tensor.transpose(tp[:], pooled[:, p * 128:p * 128 + 128],
                                        ident[:])
                    nc.scalar.copy(
                        pieces[:, p, s_i * 128:s_i * 128 + 128], tp[:])
            # ---- output stage (all chunks) ----
            if int(os.environ.get("KSTAGE", "9")) >= 5:
                AC = NCH * 128
                for l in range(4):
                    ncol = (2 * l + 1) * KMAX[l]
                    fo = pout.tile([128, NCH, 960], F32, tag="fo")
                    nc.sync.dma_start(
                        fo[:, :, :ncol],
                        din[f'featown_{l}'].ap().rearrange(
                            "(s q) c -> q s c", q=128))
                    for c0 in range(0, ncol, 128):
                        cw = min(128, ncol - c0)
                        ops_t = pp3.tile([128, 400], F32, tag="vv", bufs=2,
                                         name="ops")
                        ops = ops_t[:, 0:AC]
                        for p in range(NPIECE):
                            nc.tensor.matmul(
                                ops[:cw, :],
                                lhsT=sb[f'WU_{l}'][:, p, c0:c0 + cw],
                                rhs=pieces[:, p, :],
                                start=(p == 0), stop=(p == NPIECE - 1))
                        osb = pout.tile([128, 400], BF16, tag="osb")
                        nc.scalar.copy(osb[:cw, :AC], ops[:cw, :])
                        for s_i in range(NCH):
                            tp2 = pp3.tile([128, 128], BF16, tag="tp", bufs=2)
                            nc.tensor.transpose(
                                tp2[:, :cw],
                                osb[:cw, s_i * 128:s_i * 128 + 128],
                                ident[:cw, :cw])
                            ofin = pout.tile([128, 128], F32, tag="ofin",
                                             bufs=2)
                            nc.vector.tensor_add(out=ofin[:, :cw],
                                                 in0=tp2[:, :cw],
                                                 in1=fo[:, s_i, c0:c0 + cw])
                            nc.sync.dma_start(
                                douts[l].ap()[s_i * 128:s_i * 128 + 128,
                                              c0:c0 + cw], ofin[:, :cw])
        ctx.close()
    nc.compile()
    return nc


def kernel(**inputs):
    per_core, rep, meta = _host_prep(inputs)
    nc = build_program(meta)
    in_maps = []
    for c in range(NC_):
        m = dict(per_core[c])
        m.update(rep)
        in_maps.append(m)
    res = run_bass_kernel_spmd(nc, in_maps, list(range(NC_)))
    outs = []
    abnd = meta['abnd']
    for l in range(4):
        full = np.zeros((N_ATOMS, 2 * l + 1, KMAX[l]), np.float32)
        for c in range(NC_):
            a0, a1 = abnd[c], abnd[c + 1]
            full[a0:a1] = res.results[c][f'out_{l}'][:a1 - a0].reshape(
                a1 - a0, 2 * l + 1, KMAX[l])
        outs.append(full)
    return tuple(outs)


# revision 26
# speedup vs baseline: 1.2815x; 1.0362x over previous
"""Trainium2 Bass kernel for nn_EquivariantMessagePasser (8-core SPMD).

Strategy: edges sorted+sharded by center atom (segment-sum is core-local via
per-tile indicator matmuls into accumulating PSUM); feats replicated; per-atom
uncoupled-feature table built on device in DRAM and gathered per edge via
indirect DMA; couple-back + output linear folded into one host-precomputed
weight (WU = U x Wl).
"""
import sys

sys.path.insert(0, "/opt/trn_rl_repo")

from contextlib import ExitStack

import numpy as np
import ml_dtypes

import concourse.bass as bass
from concourse import bacc, mybir
from concourse.bass import IndirectOffsetOnAxis
from concourse.tile import TileContext
from concourse.bass_utils import run_bass_kernel_spmd
from concourse.masks import make_identity

F32 = mybir.dt.float32
BF16 = mybir.dt.bfloat16
I32 = mybir.dt.int32
bf = ml_dtypes.bfloat16

NMAX = [8, 6, 4, 2]
KMAX = [128, 96, 64, 32]
PADDED_L = [0, 2, 2, 4]
SIDE = [1, 3, 3, 5]
MSZ = [1, 4, 9, 16]
ITSZ = [1, 9, 9, 25]
N_ATOMS = 2500
HIDDEN = 64
NC_ = 8
KW = 32
LO = [96, 64, 32, 0]
KOFF = [0, 128, 224, 288]         # radial region offsets (cumsum KMAX)

# row layout: [l3 (i/t,j,k)=800 | l2 288 | l1 288 | l0 32] = 1408
GOFF = {3: 0, 2: 800, 1: 1088, 0: 1376}
GROW = 1408
PTOT = 1408
NPIECE = 11

VKB = {3: 8, 2: 8, 1: 32}         # k-channels per uncouple-V matmul
TBB = {3: 5, 2: 14, 1: 14}        # k-channels per table-build block


def _uflat(U):
    side = U.shape[0]
    return U.reshape(side * side, side * side).T.copy()  # [m, (i,j)]


def _blockdiag(mat, B):
    m, n = mat.shape
    out = np.zeros((B * m, B * n), mat.dtype)
    for b in range(B):
        out[b * m:(b + 1) * m, b * n:(b + 1) * n] = mat
    return out


def _cfeat(feats, l):
    return np.concatenate(
        [feats[lp][:, :, LO[l]:LO[l] + KW] for lp in range(l + 1)], axis=1)


def _build_wu(U, Wl):
    u0 = float(np.asarray(U[0]).reshape(-1)[0])
    wu = []
    for l in range(4):
        ncol = (2 * l + 1) * KMAX[l]
        M = np.zeros((PTOT, ncol), np.float32)
        for lch in range(l, 4):
            s = SIDE[lch]
            uf = np.asarray(U[PADDED_L[lch]], np.float32)
            koff = (lch - l) * KW
            for i in range(s):
                for j in range(s):
                    for mloc in range(2 * l + 1):
                        uv = float(uf[i, j, l * l + mloc])
                        if lch == 0:
                            uv *= u0 * u0
                        r0 = GOFF[lch] + (i * s + j) * KW
                        M[r0:r0 + KW, mloc * KMAX[l]:(mloc + 1) * KMAX[l]] += \
                            uv * Wl[l][koff:koff + KW, :]
        wu.append(M)
    return wu


def _host_prep(inp):
    rb = [np.asarray(inp[f'radial_basis_{l}'], np.float32) for l in range(4)]
    sph = [np.asarray(inp[f'spherical_harmonics_{l}'], np.float32)
           for l in range(4)]
    feats = [np.asarray(inp[f'features_{l}'], np.float32) for l in range(4)]
    centers = np.asarray(inp['centers'])
    neighbors = np.asarray(inp['neighbors'])
    U = {L: np.asarray(inp[f'U{L}'], np.float32) for L in (0, 2, 4)}
    Wr1 = [np.asarray(inp[f'Wr1_{l}'], np.float32) for l in range(4)]
    Wr2 = [np.asarray(inp[f'Wr2_{l}'], np.float32) for l in range(4)]
    Wl = [np.asarray(inp[f'Wl_{l}'], np.float32) for l in range(4)]

    order = np.argsort(centers, kind='stable')
    c_sorted = centers[order]
    abnd = [round(c * N_ATOMS / NC_) for c in range(NC_ + 1)]
    starts = np.searchsorted(c_sorted, np.arange(N_ATOMS + 1))
    core_chunks = []
    nch_max = 0
    for c in range(NC_):
        a0c, a1c = abnd[c], abnd[c + 1]
        chunks = []
        a = a0c
        while a < a1c:
            na = min(128, a1c - a)
            e0, e1 = int(starts[a]), int(starts[a + na])
            chunks.append((a, na, e0, e1 - e0))
            a += na
        core_chunks.append(chunks)
        nch_max = max(nch_max, len(chunks))
    NCH = nch_max
    for c in range(NC_):
        while len(core_chunks[c]) < NCH:
            core_chunks[c].append((abnd[c + 1], 0, 0, 0))
    nt_s = [max((core_chunks[c][s][3] + 127) // 128 for c in range(NC_))
            for s in range(NCH)]
    NT = sum(nt_s)
    EP = NT * 128
    tbase = np.cumsum([0] + nt_s)

    sph_cat = np.concatenate(sph, axis=1)

    per_core = []
    for c in range(NC_):
        eidx = np.zeros(EP, np.int64)
        valid = np.zeros(EP, np.float32)
        ind = np.zeros((NT, 128, 128), np.float32)
        for s_i, (a0, na, e0, ne) in enumerate(core_chunks[c]):
            pos0 = int(tbase[s_i]) * 128
            idx = order[e0:e0 + ne]
            eidx[pos0:pos0 + ne] = idx
            valid[pos0:pos0 + ne] = 1.0
            loc = c_sorted[e0:e0 + ne] - a0
            rows = np.arange(pos0, pos0 + ne)
            ind[rows // 128, rows % 128, loc] = 1.0
        d = {}
        for l in range(4):
            d[f'rbT_{l}'] = np.ascontiguousarray(
                (rb[l][eidx] * valid[:, None]).T).astype(bf)
        d['s_mat'] = (sph_cat[eidx] * valid[:, None]).astype(np.float32)
        d['nbr'] = np.ascontiguousarray(
            (neighbors[eidx] * valid.astype(np.int64)).astype(np.int32)
            [:, None])
        d['ind'] = ind.reshape(NT * 128, 128).astype(bf)
        a0c, a1c = abnd[c], abnd[c + 1]
        for l in range(4):
            fo = np.zeros((NCH * 128, (2 * l + 1) * KMAX[l]), np.float32)
            fo[:a1c - a0c] = feats[l][a0c:a1c].reshape(a1c - a0c, -1)
            d[f'featown_{l}'] = fo
        per_core.append(d)

    rep = {}
    for l in (1, 2, 3):
        B, m = TBB[l], MSZ[l]
        cf = _cfeat(feats, l)
        ng = (KW + B - 1) // B
        t = np.zeros((B * m, ng, N_ATOMS), np.float32)
        for g in range(ng):
            for kb in range(B):
                k = g * B + kb
                if k < KW:
                    t[kb * m:(kb + 1) * m, g, :] = cf[:, :, k].T
        rep[f'cfT_{l}'] = t.astype(bf)
        uf = _uflat(U[PADDED_L[l]])[:m, :]
        rep[f'Utab_{l}'] = _blockdiag(uf, B).astype(bf)
        rep[f'UVbd_{l}'] = _blockdiag(uf, VKB[l]).astype(bf)
    rep['g0tab'] = np.ascontiguousarray(feats[0][:, 0, 96:128]).astype(bf)
    wu = _build_wu(U, Wl)
    for l in range(4):
        rep[f'WU_{l}'] = wu[l].astype(bf)
        rep[f'Wr1_{l}'] = Wr1[l].astype(bf)
        rep[f'Wr2_{l}'] = Wr2[l].astype(bf)

    meta = dict(NT=NT, NCH=NCH, nt_s=nt_s, tbase=[int(x) for x in tbase],
                abnd=abnd, EP=EP)
    return per_core, rep, meta


def build_program(meta):
    import os
    PH = int(os.environ.get("KPHASE", "4"))
    NT, NCH, EP = meta['NT'], meta['NCH'], meta['EP']
    nt_s, tbase = meta['nt_s'], meta['tbase']

    nc = bacc.Bacc("TRN2", target_bir_lowering=False, debug=False,
                   num_devices=NC_)
    ctx = ExitStack()

    din = {}

    def dri(name, shape, dt):
        din[name] = nc.dram_tensor(name, shape, dt, kind="ExternalInput")

    for l in range(4):
        dri(f'rbT_{l}', [NMAX[l], EP], BF16)
        dri(f'featown_{l}', [NCH * 128, (2 * l + 1) * KMAX[l]], F32)
        dri(f'WU_{l}', [PTOT, (2 * l + 1) * KMAX[l]], BF16)
        dri(f'Wr1_{l}', [NMAX[l], HIDDEN], BF16)
        dri(f'Wr2_{l}', [HIDDEN, KMAX[l]], BF16)
    dri('s_mat', [EP, 16], F32)
    dri('nbr', [EP, 1], I32)
    dri('ind', [NT * 128, 128], BF16)
    for l in (1, 2, 3):
        B, m = TBB[l], MSZ[l]
        ng = (KW + B - 1) // B
        dri(f'cfT_{l}', [B * m, ng, N_ATOMS], BF16)
        dri(f'Utab_{l}', [B * m, B * SIDE[l] ** 2], BF16)
        dri(f'UVbd_{l}', [VKB[l] * m, VKB[l] * ITSZ[l]], BF16)
    dri('g0tab', [N_ATOMS, KW], BF16)
    douts = [nc.dram_tensor(f'out_{l}', [NCH * 128, (2 * l + 1) * KMAX[l]],
                            F32, kind="ExternalOutput") for l in range(4)]
    gtab = nc.dram_tensor('gtab', [N_ATOMS, GROW], BF16)

    with TileContext(nc) as tc:
        cpool = ctx.enter_context(tc.tile_pool(name="const", bufs=1))
        ident = cpool.tile([128, 128], BF16)
        make_identity(nc, ident[:])

        sb = {}
        for name in ('Utab_1', 'Utab_2', 'Utab_3', 'UVbd_1', 'UVbd_2',
                     'UVbd_3', 'Wr1_0', 'Wr1_1', 'Wr1_2', 'Wr1_3',
                     'Wr2_0', 'Wr2_1', 'Wr2_2', 'Wr2_3'):
            t = din[name]
            sb[name] = cpool.tile(list(t.shape), t.dtype, name=name)
            nc.sync.dma_start(sb[name][:], t.ap())
        for l in range(4):
            t = din[f'WU_{l}']
            w = cpool.tile([128, NPIECE, t.shape[1]], BF16, name=f'wu{l}')
            sb[f'WU_{l}'] = w
            nc.sync.dma_start(w[:],
                              t.ap().rearrange("(p q) c -> q p c", q=128))
        s_sb = cpool.tile([128, NT, 16], F32)
        nc.sync.dma_start(
            s_sb[:], din['s_mat'].ap().rearrange("(t q) m -> q t m", q=128))
        nbr_all = cpool.tile([128, NT], I32)
        nc.sync.dma_start(
            nbr_all[:], din['nbr'].ap().rearrange("(t q) one -> q (t one)",
                                                  q=128))
        ind_all = cpool.tile([128, NT, 128], BF16)
        nc.sync.dma_start(
            ind_all[:], din['ind'].ap().rearrange("(t q) a -> q t a", q=128))
        rbT_sb = {}
        for l in range(4):
            rbT_sb[l] = cpool.tile([NMAX[l], EP], BF16, name=f'rbt{l}')
            nc.sync.dma_start(rbT_sb[l][:], din[f'rbT_{l}'].ap())

        # -------- phase 1: radial MLP --------
        radial_sb = cpool.tile([128, NT, 320], BF16)
        if PH >= 1:
         with tc.tile_pool(name="p1ps", bufs=2, space="PSUM") as pp1, \
                tc.tile_pool(name="p1sb", bufs=2) as ps1:
            for l in range(4):
                for t0 in range(0, NT, 4):
                    nt4 = min(4, NT - t0)
                    ec = nt4 * 128
                    h_ps = pp1.tile([HIDDEN, 512], F32, tag="h")
                    nc.tensor.matmul(h_ps[:, :ec], lhsT=sb[f'Wr1_{l}'][:],
                                     rhs=rbT_sb[l][:, t0 * 128:t0 * 128 + ec],
                                     start=True, stop=True)
                    h_sg = ps1.tile([HIDDEN, 512], F32, tag="hsg")
                    nc.scalar.activation(h_sg[:, :ec], h_ps[:, :ec],
                                         mybir.ActivationFunctionType.Sigmoid)
                    h_sb = ps1.tile([HIDDEN, 512], BF16, tag="hs")
                    nc.vector.tensor_tensor(out=h_sb[:, :ec],
                                            in0=h_ps[:, :ec],
                                            in1=h_sg[:, :ec],
                                            op=mybir.AluOpType.mult)
                    for ti in range(nt4):
                        r_ps = pp1.tile([128, KMAX[l]], F32, tag="r")
                        nc.tensor.matmul(r_ps[:],
                                         lhsT=h_sb[:, ti * 128:ti * 128 + 128],
                                         rhs=sb[f'Wr2_{l}'][:],
                                         start=True, stop=True)
                        nc.scalar.copy(
                            radial_sb[:, t0 + ti,
                                      KOFF[l]:KOFF[l] + KMAX[l]], r_ps[:])

        # -------- phase 2: G-table build --------
        if PH >= 2:
         with tc.tile_pool(name="p2ps", bufs=2, space="PSUM") as pp2, \
                tc.tile_pool(name="p2sb", bufs=3) as ps2:
            for a0 in range(0, N_ATOMS, 512):
                ac = min(512, N_ATOMS - a0)
                nq = (ac + 127) // 128
                grows = ps2.tile([128, 4, GOFF[0]], BF16, tag="grows")
                for l in (3, 2, 1):
                    B, m, s2 = TBB[l], MSZ[l], SIDE[l] ** 2
                    ng = (KW + B - 1) // B
                    cf = ps2.tile([B * m, ng, 512], BF16, tag=f"cf{l}")
                    nc.sync.dma_start(cf[:, :, :ac],
                                      din[f'cfT_{l}'].ap()[:, :, a0:a0 + ac])
                    for g in range(ng):
                        nkb = min(B, KW - g * B)
                        gt_ps = pp2.tile([B * s2, 512], F32, tag="gt")
                        nc.tensor.matmul(gt_ps[:, :ac],
                                         lhsT=sb[f'Utab_{l}'][:],
                                         rhs=cf[:, g, :ac],
                                         start=True, stop=True)
                        gt_sb = ps2.tile([B * s2, 512], BF16, tag="gts")
                        nc.scalar.copy(gt_sb[:, :ac], gt_ps[:, :ac])
                        for q in range(nq):
                            an = min(128, ac - q * 128)
                            tp = pp2.tile([128, B * s2], BF16, tag="tp")
                            nc.tensor.transpose(
                                tp[:an, :], gt_sb[:, q * 128:q * 128 + an],
                                ident[:B * s2, :B * s2])
                            dst = grows[:an, q, :].rearrange(
                                "p (tj k) -> p tj k", k=KW)[
                                :, GOFF[l] // KW:GOFF[l] // KW + s2,
                                g * B:g * B + nkb]
                            src = tp[:an, :].rearrange(
                                "p (kb tj) -> p kb tj", kb=B)[
                                :, :nkb, :].rearrange("p kb tj -> p tj kb")
                            nc.vector.tensor_copy(dst, src)
                g0s = ps2.tile([128, 4, KW], BF16, tag="g0s")
                for q in range(nq):
                    an = min(128, ac - q * 128)
                    nc.sync.dma_start(
                        g0s[:an, q, :],
                        din['g0tab'].ap()[a0 + q * 128:a0 + q * 128 + an, :])
                    nc.sync.dma_start(
                        gtab.ap()[a0 + q * 128:a0 + q * 128 + an, :GOFF[0]],
                        grows[:an, q, :])
                    nc.sync.dma_start(
                        gtab.ap()[a0 + q * 128:a0 + q * 128 + an,
                                  GOFF[0]:GOFF[0] + KW],
                        g0s[:an, q, :])

        # -------- phase 3: edge loop --------
        coff = {3: 0, 2: 512, 1: 800, 0: 928}
        if PH >= 3:
         with tc.tile_pool(name="plps", bufs=1, space="PSUM") as poolp, \
                tc.tile_pool(name="p3ps", bufs=1, space="PSUM") as pp3, \
                tc.tile_pool(name="p3sb", bufs=3) as ps3, \
                tc.tile_pool(name="pout", bufs=1) as pout:
            pieces = pout.tile([128, NPIECE, NCH * 128], BF16,
                               tag="pieces", name="pieces")
            for s_i in range(NCH):
                np3 = [poolp.tile([128, 400], F32, tag="pl3a", name="pl3a"),
                       poolp.tile([128, 400], F32, tag="pl3b", name="pl3b")]
                np2 = poolp.tile([128, 288], F32, tag="pl2", name="pl2")
                np10 = poolp.tile([128, 320], F32, tag="pl10", name="pl10")
                ntl = min(nt_s[s_i], int(os.environ.get("KTILES", "999")))
                for tloc in range(ntl):
                    ti = tbase[s_i] + tloc
                    first = tloc == 0
                    last = tloc == ntl - 1
                    # cvec
                    cvec = ps3.tile([128, 960], BF16, tag="cvec")
                    for l in range(4):
                        for lp in range(l + 1):
                            mlo, msz = lp * lp, 2 * lp + 1
                            dst = cvec[:, coff[l]:coff[l] + KW * MSZ[l]]\
                                .rearrange("p (k m) -> p k m", k=KW)\
                                [:, :, mlo:mlo + msz]
                            s_in = s_sb[:, ti, mlo:mlo + msz].unsqueeze(1)\
                                .broadcast_to([128, KW, msz])
                            r_in = radial_sb[:, ti, KOFF[lp] + LO[l]:
                                             KOFF[lp] + LO[l] + KW]\
                                .unsqueeze(2).broadcast_to([128, KW, msz])
                            nc.vector.tensor_tensor(out=dst, in0=s_in,
                                                    in1=r_in,
                                                    op=mybir.AluOpType.mult)
                    # transposes
                    KS = int(os.environ.get("KSTAGE", "9"))
                    cvT = {3: [], 2: [], 1: []}
                    blocks = [(3, 0, 128), (3, 128, 128), (3, 256, 128),
                              (3, 384, 128),
                              (2, 512, 72), (2, 584, 72),
                              (2, 656, 72), (2, 728, 72),
                              (1, 800, 128)]
                    for bi, (l, off, w) in enumerate(blocks) if KS >= 2 else []:
                        tp = pp3.tile([128, 128], BF16, tag="tp", bufs=2)
                        nc.tensor.transpose(tp[:w, :], cvec[:, off:off + w],
                                            ident[:])
                        piece = ps3.tile([128, 128], BF16, tag=f"cvT{bi}", name=f"cvT{bi}")
                        nc.scalar.copy(piece[:w, :], tp[:w, :])
                        cvT[l].append(piece)
                    # uncouple V -> vsb (it,k) layout bf16
                    vsb3 = ps3.tile([128, 800], BF16, tag="vs3", name="vs3")
                    vsb = {2: ps3.tile([128, 288], BF16, tag="vs2", name="vs2"),
                           1: ps3.tile([128, 288], BF16, tag="vs1", name="vs1")}
                    _lset = tuple(int(x) for x in os.environ.get(
                        "KLSET", "321"))
                    for l in (_lset if KS >= 3 else []):
                        nkb, m, it = VKB[l], MSZ[l], ITSZ[l]
                        if l == 3:
                            for h in range(2):
                                vp = pp3.tile([128, 400], F32, tag="vv", bufs=2,
                                              name="vv")
                                for qq in range(2):
                                    q = h * 2 + qq
                                    nc.tensor.matmul(
                                        vp[:, qq * 200:qq * 200 + 200],
                                        lhsT=cvT[3][q][:128, :],
                                        rhs=sb['UVbd_3'][:],
                                        start=True, stop=True)
                                if "KNOCOPY" not in os.environ:
                                    srcap = vp[:].rearrange(
                                        "p (kq kk it) -> p kq kk it",
                                        kq=2, kk=8)\
                                        .rearrange("p kq kk it -> p kq it kk")
                                    dst = vsb3[:].rearrange(
                                        "p (it k) -> p it k", k=KW)\
                                        [:, :, h * 16:h * 16 + 16].rearrange(
                                        "p it (kq kk) -> p kq it kk", kq=2)
                                    nc.scalar.copy(dst, srcap)
                        elif l == 2:
                            vp = pp3.tile([128, 288], F32, tag="vv", bufs=2, name="vv")
                            for q in range(4):
                                nc.tensor.matmul(
                                    vp[:, q * 72:q * 72 + 72],
                                    lhsT=cvT[2][q][:72, :],
                                    rhs=sb['UVbd_2'][:],
                                    start=True, stop=True)
                            if "KNOCOPY" not in os.environ:
                                srcap = vp[:].rearrange(
                                    "p (kq kk it) -> p kq kk it", kq=4, kk=8)\
                                    .rearrange("p kq kk it -> p kq it kk")
                                dst = vsb[2][:].rearrange(
                                    "p (it kq kk) -> p kq it kk", kq=4, kk=8)
                                nc.scalar.copy(dst, srcap)
                        else:
                            vp = pp3.tile([128, 288], F32, tag="vv", bufs=2, name="vv")
                            nc.tensor.matmul(vp[:], lhsT=cvT[1][0][:128, :],
                                             rhs=sb['UVbd_1'][:],
                                             start=True, stop=True)
                            if "KNOCOPY" not in os.environ:
                                srcap = vp[:].rearrange(
                                    "p (kk it) -> p kk it", kk=32)\
                                    .rearrange("p kk it -> p it kk")
                                dst = vsb[1][:].rearrange(
                                    "p (it kk) -> p it kk", kk=32)
                                nc.scalar.copy(dst, srcap)
                    # gather
                    G = ps3.tile([128, GROW], BF16, tag="G")
                    if PH >= 4:
                        nc.gpsimd.indirect_dma_start(
                            out=G[:], out_offset=None, in_=gtab.ap()[:, :],
                            in_offset=IndirectOffsetOnAxis(
                                ap=nbr_all[:, ti:ti + 1], axis=0))
                    else:
                        nc.gpsimd.memset(G[:], 0.0)
                    # products
                    PT = [ps3.tile([128, 1408], BF16, tag=f"PT{t}", name=f"PT{t}")
                          for t in range(5)]
                    for t in range(5) if KS >= 4 else []:
                        v = vsb3[:].rearrange(
                            "p (i t k) -> p i t k", i=5, k=KW)\
                            [:, :, t, :].unsqueeze(2)\
                            .broadcast_to([128, 5, 5, KW])
                        g3 = G[:, GOFF[3]:GOFF[3] + 800].rearrange(
                            "p (t j k) -> p t j k", t=5, k=KW)\
                            [:, t, :, :].unsqueeze(1)\
                            .broadcast_to([128, 5, 5, KW])
                        o = PT[t][:, 0:800].rearrange(
                            "p (i j k) -> p i j k", i=5, k=KW)
                        nc.vector.tensor_tensor(out=o, in0=v, in1=g3,
                                                op=mybir.AluOpType.mult)
                    for l in (2, 1) if KS >= 4 else []:
                        for t in range(3):
                            v = vsb[l][:].rearrange(
                                "p (i t k) -> p i t k", i=3, k=KW)\
                                [:, :, t, :].unsqueeze(2)\
                                .broadcast_to([128, 3, 3, KW])
                            gl = G[:, GOFF[l]:GOFF[l] + 288].rearrange(
                                "p (t j k) -> p t j k", t=3, k=KW)\
                                [:, t, :, :].unsqueeze(1)\
                                .broadcast_to([128, 3, 3, KW])
                            o = PT[t][:, GOFF[l]:GOFF[l] + 288]\
                                .rearrange("p (i j k) -> p i j k", i=3, k=KW)
                            nc.vector.tensor_tensor(out=o, in0=v, in1=gl,
                                                    op=mybir.AluOpType.mult)
                    if KS >= 4:
                     nc.vector.tensor_tensor(
                        out=PT[0][:, 1376:1408], in0=cvec[:, 928:960],
                        in1=G[:, GOFF[0]:GOFF[0] + KW],
                        op=mybir.AluOpType.mult)
                    # segment matmuls
                    if KS < 5:
                        continue
                    lhs_ind = ind_all[:, ti, :]
                    for t in range(5):
                        for h in range(2):
                            nc.tensor.matmul(
                                np3[h][:], lhsT=lhs_ind,
                                rhs=PT[t][:, h * 400:h * 400 + 400],
                                start=(first and t == 0),
                                stop=(last and t == 4))
                    for t in range(3):
                        nc.tensor.matmul(np2[:], lhsT=lhs_ind,
                                         rhs=PT[t][:, 800:1088],
                                         start=(first and t == 0),
                                         stop=(last and t == 2))
                    nc.tensor.matmul(np10[:], lhsT=lhs_ind,
                                     rhs=PT[0][:, 1088:1408],
                                     start=first, stop=False)
                    for t in (1, 2):
                        nc.tensor.matmul(np10[:, :288], lhsT=lhs_ind,
                                         rhs=PT[t][:, 1088:1376],
                                         start=False, stop=(last and t == 2))
                # ---- chunk epilogue ----
                if int(os.environ.get("KSTAGE", "9")) < 5:
                    continue
                pooled = pout.tile([128, PTOT], BF16, tag="pooled")
                nc.scalar.copy(pooled[:, 0:400], np3[0][:])
                nc.scalar.copy(pooled[:, 400:800], np3[1][:])
                nc.scalar.copy(pooled[:, 800:1088], np2[:])
                nc.scalar.copy(pooled[:, 1088:1408], np10[:])
                for p in range(NPIECE):
                    tp = pp3.tile([128, 128], BF16, tag="tp", bufs=2)
                    nc.tensor.transpose(tp[:], pooled[:, p * 128:p * 128 + 128],
                                        ident[:])
                    nc.scalar.copy(
                        pieces[:, p, s_i * 128:s_i * 128 + 128], tp[:])
            # ---- output stage (all chunks) ----
            if int(os.environ.get("KSTAGE", "9")) >= 5:
                AC = NCH * 128
                for l in range(4):
                    ncol = (2 * l + 1) * KMAX[l]
                    fo = pout.tile([128, NCH, 960], F32, tag="fo")
                    nc.sync.dma_start(
                        fo[:, :, :ncol],
                        din[f'featown_{l}'].ap().rearrange(
                            "(s q) c -> q s c", q=128))
                    for c0 in range(0, ncol, 128):
                        cw = min(128, ncol - c0)
                        ops_t = pp3.tile([128, 400], F32, tag="vv", bufs=2,
                                         name="ops")
                        ops = ops_t[:, 0:AC]
                        for p in range(NPIECE):
                            nc.tensor.matmul(
                                ops[:cw, :],
                                lhsT=sb[f'WU_{l}'][:, p, c0:c0 + cw],
                                rhs=pieces[:, p, :],
                                start=(p == 0), stop=(p == NPIECE - 1))
                        osb = pout.tile([128, 400], BF16, tag="osb")
                        nc.scalar.copy(osb[:cw, :AC], ops[:cw, :])
                        for s_i in range(NCH):
                            tp2 = pp3.tile([128, 128], BF16, tag="tp", bufs=2)
                            nc.tensor.transpose(
                                tp2[:, :cw],
                                osb[:cw, s_i * 128:s_i * 128 + 128],
                                ident[:cw, :cw])
                            ofin = pout.tile([128, 128], F32, tag="ofin",
                                             bufs=2)
                            nc.vector.tensor_add(out=ofin[:, :cw],
                                                 in0=tp2[:, :cw],
                                                 in1=fo[:, s_i, c0:c0 + cw])
                            nc.sync.dma_start(
                                douts[l].ap()[s_i * 128:s_i * 128 + 128,
                                              c0:c0 + cw], ofin[:, :cw])
        ctx.close()
    nc.compile()
    return nc


def kernel(**inputs):
    per_core, rep, meta = _host_prep(inputs)
    nc = build_program(meta)
    in_maps = []
    for c in range(NC_):
        m = dict(per_core[c])
        m.update(rep)
        in_maps.append(m)
    res = run_bass_kernel_spmd(nc, in_maps, list(range(NC_)))
    outs = []
    abnd = meta['abnd']
    for l in range(4):
        full = np.zeros((N_ATOMS, 2 * l + 1, KMAX[l]), np.float32)
        for c in range(NC_):
            a0, a1 = abnd[c], abnd[c + 1]
            full[a0:a1] = res.results[c][f'out_{l}'][:a1 - a0].reshape(
                a1 - a0, 2 * l + 1, KMAX[l])
        outs.append(full)
    return tuple(outs)


# revision 27
# speedup vs baseline: 1.3069x; 1.0198x over previous
"""Trainium2 Bass kernel for nn_EquivariantMessagePasser (8-core SPMD).

Strategy: edges sorted+sharded by center atom (segment-sum is core-local via
per-tile indicator matmuls into accumulating PSUM); feats replicated; per-atom
uncoupled-feature table built on device in DRAM and gathered per edge via
indirect DMA; couple-back + output linear folded into one host-precomputed
weight (WU = U x Wl).
"""
import sys

sys.path.insert(0, "/opt/trn_rl_repo")

from contextlib import ExitStack

import numpy as np
import ml_dtypes

import concourse.bass as bass
from concourse import bacc, mybir
from concourse.bass import IndirectOffsetOnAxis
from concourse.tile import TileContext
from concourse.bass_utils import run_bass_kernel_spmd
from concourse.masks import make_identity

F32 = mybir.dt.float32
BF16 = mybir.dt.bfloat16
I32 = mybir.dt.int32
bf = ml_dtypes.bfloat16

NMAX = [8, 6, 4, 2]
KMAX = [128, 96, 64, 32]
PADDED_L = [0, 2, 2, 4]
SIDE = [1, 3, 3, 5]
MSZ = [1, 4, 9, 16]
ITSZ = [1, 9, 9, 25]
N_ATOMS = 2500
HIDDEN = 64
NC_ = 8
KW = 32
LO = [96, 64, 32, 0]
KOFF = [0, 128, 224, 288]         # radial region offsets (cumsum KMAX)

# row layout: [l3 (i/t,j,k)=800 | l2 288 | l1 288 | l0 32] = 1408
GOFF = {3: 0, 2: 800, 1: 1088, 0: 1376}
GROW = 1408
PTOT = 1408
NPIECE = 11

VKB = {3: 8, 2: 14, 1: 32}         # k-channels per uncouple-V matmul
TBB = {3: 5, 2: 14, 1: 14}        # k-channels per table-build block


def _uflat(U):
    side = U.shape[0]
    return U.reshape(side * side, side * side).T.copy()  # [m, (i,j)]


def _blockdiag(mat, B):
    m, n = mat.shape
    out = np.zeros((B * m, B * n), mat.dtype)
    for b in range(B):
        out[b * m:(b + 1) * m, b * n:(b + 1) * n] = mat
    return out


def _cfeat(feats, l):
    return np.concatenate(
        [feats[lp][:, :, LO[l]:LO[l] + KW] for lp in range(l + 1)], axis=1)


def _build_wu(U, Wl):
    u0 = float(np.asarray(U[0]).reshape(-1)[0])
    wu = []
    for l in range(4):
        ncol = (2 * l + 1) * KMAX[l]
        M = np.zeros((PTOT, ncol), np.float32)
        for lch in range(l, 4):
            s = SIDE[lch]
            uf = np.asarray(U[PADDED_L[lch]], np.float32)
            koff = (lch - l) * KW
            for i in range(s):
                for j in range(s):
                    for mloc in range(2 * l + 1):
                        uv = float(uf[i, j, l * l + mloc])
                        if lch == 0:
                            uv *= u0 * u0
                        r0 = GOFF[lch] + (i * s + j) * KW
                        M[r0:r0 + KW, mloc * KMAX[l]:(mloc + 1) * KMAX[l]] += \
                            uv * Wl[l][koff:koff + KW, :]
        wu.append(M)
    return wu


def _host_prep(inp):
    rb = [np.asarray(inp[f'radial_basis_{l}'], np.float32) for l in range(4)]
    sph = [np.asarray(inp[f'spherical_harmonics_{l}'], np.float32)
           for l in range(4)]
    feats = [np.asarray(inp[f'features_{l}'], np.float32) for l in range(4)]
    centers = np.asarray(inp['centers'])
    neighbors = np.asarray(inp['neighbors'])
    U = {L: np.asarray(inp[f'U{L}'], np.float32) for L in (0, 2, 4)}
    Wr1 = [np.asarray(inp[f'Wr1_{l}'], np.float32) for l in range(4)]
    Wr2 = [np.asarray(inp[f'Wr2_{l}'], np.float32) for l in range(4)]
    Wl = [np.asarray(inp[f'Wl_{l}'], np.float32) for l in range(4)]

    order = np.argsort(centers, kind='stable')
    c_sorted = centers[order]
    abnd = [round(c * N_ATOMS / NC_) for c in range(NC_ + 1)]
    starts = np.searchsorted(c_sorted, np.arange(N_ATOMS + 1))
    core_chunks = []
    nch_max = 0
    for c in range(NC_):
        a0c, a1c = abnd[c], abnd[c + 1]
        chunks = []
        a = a0c
        while a < a1c:
            na = min(128, a1c - a)
            e0, e1 = int(starts[a]), int(starts[a + na])
            chunks.append((a, na, e0, e1 - e0))
            a += na
        core_chunks.append(chunks)
        nch_max = max(nch_max, len(chunks))
    NCH = nch_max
    for c in range(NC_):
        while len(core_chunks[c]) < NCH:
            core_chunks[c].append((abnd[c + 1], 0, 0, 0))
    nt_s = [max((core_chunks[c][s][3] + 127) // 128 for c in range(NC_))
            for s in range(NCH)]
    NT = sum(nt_s)
    EP = NT * 128
    tbase = np.cumsum([0] + nt_s)

    sph_cat = np.concatenate(sph, axis=1)

    per_core = []
    for c in range(NC_):
        eidx = np.zeros(EP, np.int64)
        valid = np.zeros(EP, np.float32)
        ind = np.zeros((NT, 128, 128), np.float32)
        for s_i, (a0, na, e0, ne) in enumerate(core_chunks[c]):
            pos0 = int(tbase[s_i]) * 128
            idx = order[e0:e0 + ne]
            eidx[pos0:pos0 + ne] = idx
            valid[pos0:pos0 + ne] = 1.0
            loc = c_sorted[e0:e0 + ne] - a0
            rows = np.arange(pos0, pos0 + ne)
            ind[rows // 128, rows % 128, loc] = 1.0
        d = {}
        for l in range(4):
            d[f'rbT_{l}'] = np.ascontiguousarray(
                (rb[l][eidx] * valid[:, None]).T).astype(bf)
        d['s_mat'] = (sph_cat[eidx] * valid[:, None]).astype(np.float32)
        d['nbr'] = np.ascontiguousarray(
            (neighbors[eidx] * valid.astype(np.int64)).astype(np.int32)
            [:, None])
        d['ind'] = ind.reshape(NT * 128, 128).astype(bf)
        a0c, a1c = abnd[c], abnd[c + 1]
        for l in range(4):
            fo = np.zeros((NCH * 128, (2 * l + 1) * KMAX[l]), np.float32)
            fo[:a1c - a0c] = feats[l][a0c:a1c].reshape(a1c - a0c, -1)
            d[f'featown_{l}'] = fo
        per_core.append(d)

    rep = {}
    for l in (1, 2, 3):
        B, m = TBB[l], MSZ[l]
        cf = _cfeat(feats, l)
        ng = (KW + B - 1) // B
        t = np.zeros((B * m, ng, N_ATOMS), np.float32)
        for g in range(ng):
            for kb in range(B):
                k = g * B + kb
                if k < KW:
                    t[kb * m:(kb + 1) * m, g, :] = cf[:, :, k].T
        rep[f'cfT_{l}'] = t.astype(bf)
        uf = _uflat(U[PADDED_L[l]])[:m, :]
        rep[f'Utab_{l}'] = _blockdiag(uf, B).astype(bf)
        rep[f'UVbd_{l}'] = _blockdiag(uf, VKB[l]).astype(bf)
    rep['g0tab'] = np.ascontiguousarray(feats[0][:, 0, 96:128]).astype(bf)
    wu = _build_wu(U, Wl)
    for l in range(4):
        rep[f'WU_{l}'] = wu[l].astype(bf)
        rep[f'Wr1_{l}'] = Wr1[l].astype(bf)
        rep[f'Wr2_{l}'] = Wr2[l].astype(bf)

    meta = dict(NT=NT, NCH=NCH, nt_s=nt_s, tbase=[int(x) for x in tbase],
                abnd=abnd, EP=EP)
    return per_core, rep, meta


def build_program(meta):
    import os
    PH = int(os.environ.get("KPHASE", "4"))
    NT, NCH, EP = meta['NT'], meta['NCH'], meta['EP']
    nt_s, tbase = meta['nt_s'], meta['tbase']

    nc = bacc.Bacc("TRN2", target_bir_lowering=False, debug=False,
                   num_devices=NC_)
    ctx = ExitStack()

    din = {}

    def dri(name, shape, dt):
        din[name] = nc.dram_tensor(name, shape, dt, kind="ExternalInput")

    for l in range(4):
        dri(f'rbT_{l}', [NMAX[l], EP], BF16)
        dri(f'featown_{l}', [NCH * 128, (2 * l + 1) * KMAX[l]], F32)
        dri(f'WU_{l}', [PTOT, (2 * l + 1) * KMAX[l]], BF16)
        dri(f'Wr1_{l}', [NMAX[l], HIDDEN], BF16)
        dri(f'Wr2_{l}', [HIDDEN, KMAX[l]], BF16)
    dri('s_mat', [EP, 16], F32)
    dri('nbr', [EP, 1], I32)
    dri('ind', [NT * 128, 128], BF16)
    for l in (1, 2, 3):
        B, m = TBB[l], MSZ[l]
        ng = (KW + B - 1) // B
        dri(f'cfT_{l}', [B * m, ng, N_ATOMS], BF16)
        dri(f'Utab_{l}', [B * m, B * SIDE[l] ** 2], BF16)
        dri(f'UVbd_{l}', [VKB[l] * m, VKB[l] * ITSZ[l]], BF16)
    dri('g0tab', [N_ATOMS, KW], BF16)
    douts = [nc.dram_tensor(f'out_{l}', [NCH * 128, (2 * l + 1) * KMAX[l]],
                            F32, kind="ExternalOutput") for l in range(4)]
    gtab = nc.dram_tensor('gtab', [N_ATOMS, GROW], BF16)

    with TileContext(nc) as tc:
        cpool = ctx.enter_context(tc.tile_pool(name="const", bufs=1))
        ident = cpool.tile([128, 128], BF16)
        make_identity(nc, ident[:])

        sb = {}
        for name in ('Utab_1', 'Utab_2', 'Utab_3', 'UVbd_1', 'UVbd_2',
                     'UVbd_3', 'Wr1_0', 'Wr1_1', 'Wr1_2', 'Wr1_3',
                     'Wr2_0', 'Wr2_1', 'Wr2_2', 'Wr2_3'):
            t = din[name]
            sb[name] = cpool.tile(list(t.shape), t.dtype, name=name)
            nc.sync.dma_start(sb[name][:], t.ap())
        for l in range(4):
            t = din[f'WU_{l}']
            w = cpool.tile([128, NPIECE, t.shape[1]], BF16, name=f'wu{l}')
            sb[f'WU_{l}'] = w
            nc.sync.dma_start(w[:],
                              t.ap().rearrange("(p q) c -> q p c", q=128))
        s_sb = cpool.tile([128, NT, 16], F32)
        nc.sync.dma_start(
            s_sb[:], din['s_mat'].ap().rearrange("(t q) m -> q t m", q=128))
        nbr_all = cpool.tile([128, NT], I32)
        nc.sync.dma_start(
            nbr_all[:], din['nbr'].ap().rearrange("(t q) one -> q (t one)",
                                                  q=128))
        ind_all = cpool.tile([128, NT, 128], BF16)
        nc.sync.dma_start(
            ind_all[:], din['ind'].ap().rearrange("(t q) a -> q t a", q=128))
        rbT_sb = {}
        for l in range(4):
            rbT_sb[l] = cpool.tile([NMAX[l], EP], BF16, name=f'rbt{l}')
            nc.sync.dma_start(rbT_sb[l][:], din[f'rbT_{l}'].ap())

        # -------- phase 1: radial MLP --------
        radial_sb = cpool.tile([128, NT, 320], BF16)
        if PH >= 1:
         with tc.tile_pool(name="p1ps", bufs=2, space="PSUM") as pp1, \
                tc.tile_pool(name="p1sb", bufs=2) as ps1:
            for l in range(4):
                for t0 in range(0, NT, 4):
                    nt4 = min(4, NT - t0)
                    ec = nt4 * 128
                    h_ps = pp1.tile([HIDDEN, 512], F32, tag="h")
                    nc.tensor.matmul(h_ps[:, :ec], lhsT=sb[f'Wr1_{l}'][:],
                                     rhs=rbT_sb[l][:, t0 * 128:t0 * 128 + ec],
                                     start=True, stop=True)
                    h_sg = ps1.tile([HIDDEN, 512], F32, tag="hsg")
                    nc.scalar.activation(h_sg[:, :ec], h_ps[:, :ec],
                                         mybir.ActivationFunctionType.Sigmoid)
                    h_sb = ps1.tile([HIDDEN, 512], BF16, tag="hs")
                    nc.vector.tensor_tensor(out=h_sb[:, :ec],
                                            in0=h_ps[:, :ec],
                                            in1=h_sg[:, :ec],
                                            op=mybir.AluOpType.mult)
                    for ti in range(nt4):
                        r_ps = pp1.tile([128, KMAX[l]], F32, tag="r")
                        nc.tensor.matmul(r_ps[:],
                                         lhsT=h_sb[:, ti * 128:ti * 128 + 128],
                                         rhs=sb[f'Wr2_{l}'][:],
                                         start=True, stop=True)
                        nc.scalar.copy(
                            radial_sb[:, t0 + ti,
                                      KOFF[l]:KOFF[l] + KMAX[l]], r_ps[:])

        # -------- phase 2: G-table build --------
        if PH >= 2:
         with tc.tile_pool(name="p2ps", bufs=2, space="PSUM") as pp2, \
                tc.tile_pool(name="p2sb", bufs=3) as ps2:
            for a0 in range(0, N_ATOMS, 512):
                ac = min(512, N_ATOMS - a0)
                nq = (ac + 127) // 128
                grows = ps2.tile([128, 4, GOFF[0]], BF16, tag="grows")
                for l in (3, 2, 1):
                    B, m, s2 = TBB[l], MSZ[l], SIDE[l] ** 2
                    ng = (KW + B - 1) // B
                    cf = ps2.tile([B * m, ng, 512], BF16, tag=f"cf{l}")
                    nc.sync.dma_start(cf[:, :, :ac],
                                      din[f'cfT_{l}'].ap()[:, :, a0:a0 + ac])
                    for g in range(ng):
                        nkb = min(B, KW - g * B)
                        gt_ps = pp2.tile([B * s2, 512], F32, tag="gt")
                        nc.tensor.matmul(gt_ps[:, :ac],
                                         lhsT=sb[f'Utab_{l}'][:],
                                         rhs=cf[:, g, :ac],
                                         start=True, stop=True)
                        gt_sb = ps2.tile([B * s2, 512], BF16, tag="gts")
                        nc.scalar.copy(gt_sb[:, :ac], gt_ps[:, :ac])
                        for q in range(nq):
                            an = min(128, ac - q * 128)
                            tp = pp2.tile([128, B * s2], BF16, tag="tp")
                            nc.tensor.transpose(
                                tp[:an, :], gt_sb[:, q * 128:q * 128 + an],
                                ident[:B * s2, :B * s2])
                            dst = grows[:an, q, :].rearrange(
                                "p (tj k) -> p tj k", k=KW)[
                                :, GOFF[l] // KW:GOFF[l] // KW + s2,
                                g * B:g * B + nkb]
                            src = tp[:an, :].rearrange(
                                "p (kb tj) -> p kb tj", kb=B)[
                                :, :nkb, :].rearrange("p kb tj -> p tj kb")
                            nc.vector.tensor_copy(dst, src)
                g0s = ps2.tile([128, 4, KW], BF16, tag="g0s")
                for q in range(nq):
                    an = min(128, ac - q * 128)
                    nc.sync.dma_start(
                        g0s[:an, q, :],
                        din['g0tab'].ap()[a0 + q * 128:a0 + q * 128 + an, :])
                    nc.sync.dma_start(
                        gtab.ap()[a0 + q * 128:a0 + q * 128 + an, :GOFF[0]],
                        grows[:an, q, :])
                    nc.sync.dma_start(
                        gtab.ap()[a0 + q * 128:a0 + q * 128 + an,
                                  GOFF[0]:GOFF[0] + KW],
                        g0s[:an, q, :])

        # -------- phase 3: edge loop --------
        coff = {3: 0, 2: 512, 1: 800, 0: 928}
        if PH >= 3:
         with tc.tile_pool(name="plps", bufs=1, space="PSUM") as poolp, \
                tc.tile_pool(name="p3ps", bufs=1, space="PSUM") as pp3, \
                tc.tile_pool(name="p3sb", bufs=3) as ps3, \
                tc.tile_pool(name="pout", bufs=1) as pout:
            pieces = pout.tile([128, NPIECE, NCH * 128], BF16,
                               tag="pieces", name="pieces")
            for s_i in range(NCH):
                np3 = [poolp.tile([128, 400], F32, tag="pl3a", name="pl3a"),
                       poolp.tile([128, 400], F32, tag="pl3b", name="pl3b")]
                np2 = poolp.tile([128, 288], F32, tag="pl2", name="pl2")
                np10 = poolp.tile([128, 320], F32, tag="pl10", name="pl10")
                ntl = min(nt_s[s_i], int(os.environ.get("KTILES", "999")))
                for tloc in range(ntl):
                    ti = tbase[s_i] + tloc
                    first = tloc == 0
                    last = tloc == ntl - 1
                    # cvec
                    cvec = ps3.tile([128, 960], BF16, tag="cvec")
                    for l in range(4):
                        for lp in range(l + 1):
                            mlo, msz = lp * lp, 2 * lp + 1
                            dst = cvec[:, coff[l]:coff[l] + KW * MSZ[l]]\
                                .rearrange("p (k m) -> p k m", k=KW)\
                                [:, :, mlo:mlo + msz]
                            s_in = s_sb[:, ti, mlo:mlo + msz].unsqueeze(1)\
                                .broadcast_to([128, KW, msz])
                            r_in = radial_sb[:, ti, KOFF[lp] + LO[l]:
                                             KOFF[lp] + LO[l] + KW]\
                                .unsqueeze(2).broadcast_to([128, KW, msz])
                            nc.vector.tensor_tensor(out=dst, in0=s_in,
                                                    in1=r_in,
                                                    op=mybir.AluOpType.mult)
                    # transposes
                    KS = int(os.environ.get("KSTAGE", "9"))
                    cvT = {3: [], 2: [], 1: []}
                    blocks = [(3, 0, 128), (3, 128, 128), (3, 256, 128),
                              (3, 384, 128),
                              (2, 512, 126), (2, 638, 126), (2, 764, 36),
                              (1, 800, 128)]
                    for bi, (l, off, w) in enumerate(blocks) if KS >= 2 else []:
                        tp = pp3.tile([128, 128], BF16, tag="tp", bufs=2)
                        nc.tensor.transpose(tp[:w, :], cvec[:, off:off + w],
                                            ident[:])
                        piece = ps3.tile([128, 128], BF16, tag=f"cvT{bi}", name=f"cvT{bi}")
                        nc.scalar.copy(piece[:w, :], tp[:w, :])
                        cvT[l].append(piece)
                    # uncouple V -> vsb (it,k) layout bf16
                    vsb3 = ps3.tile([128, 800], BF16, tag="vs3", name="vs3")
                    vsb = {2: ps3.tile([128, 288], BF16, tag="vs2", name="vs2"),
                           1: ps3.tile([128, 288], BF16, tag="vs1", name="vs1")}
                    _lset = tuple(int(x) for x in os.environ.get(
                        "KLSET", "321"))
                    for l in (_lset if KS >= 3 else []):
                        nkb, m, it = VKB[l], MSZ[l], ITSZ[l]
                        if l == 3:
                            for h in range(2):
                                vp = pp3.tile([128, 400], F32, tag="vv", bufs=2,
                                              name="vv")
                                for qq in range(2):
                                    q = h * 2 + qq
                                    nc.tensor.matmul(
                                        vp[:, qq * 200:qq * 200 + 200],
                                        lhsT=cvT[3][q][:128, :],
                                        rhs=sb['UVbd_3'][:],
                                        start=True, stop=True)
                                if "KNOCOPY" not in os.environ:
                                    srcap = vp[:].rearrange(
                                        "p (kq kk it) -> p kq kk it",
                                        kq=2, kk=8)\
                                        .rearrange("p kq kk it -> p kq it kk")
                                    dst = vsb3[:].rearrange(
                                        "p (it k) -> p it k", k=KW)\
                                        [:, :, h * 16:h * 16 + 16].rearrange(
                                        "p it (kq kk) -> p kq it kk", kq=2)
                                    nc.scalar.copy(dst, srcap)
                        elif l == 2:
                            vp = pp3.tile([128, 288], F32, tag="vv", bufs=2, name="vv")
                            for q in range(2):
                                nc.tensor.matmul(
                                    vp[:, q * 126:q * 126 + 126],
                                    lhsT=cvT[2][q][:126, :],
                                    rhs=sb['UVbd_2'][:],
                                    start=True, stop=True)
                            nc.tensor.matmul(
                                vp[:, 252:288],
                                lhsT=cvT[2][2][:36, :],
                                rhs=sb['UVbd_2'][:36, :36],
                                start=True, stop=True)
                            if "KNOCOPY" not in os.environ:
                                srcap = vp[:, 0:252].rearrange(
                                    "p (kq kk it) -> p kq kk it", kq=2, kk=14)\
                                    .rearrange("p kq kk it -> p kq it kk")
                                dst = vsb[2][:].rearrange(
                                    "p (it k) -> p it k", k=KW)\
                                    [:, :, 0:28].rearrange(
                                    "p it (kq kk) -> p kq it kk", kq=2)
                                nc.scalar.copy(dst, srcap)
                                srcb = vp[:, 252:288].rearrange(
                                    "p (kk it) -> p kk it", kk=4)\
                                    .rearrange("p kk it -> p it kk")
                                dstb = vsb[2][:].rearrange(
                                    "p (it k) -> p it k", k=KW)[:, :, 28:32]
                                nc.scalar.copy(dstb, srcb)
                        else:
                            vp = pp3.tile([128, 288], F32, tag="vv", bufs=2, name="vv")
                            nc.tensor.matmul(vp[:], lhsT=cvT[1][0][:128, :],
                                             rhs=sb['UVbd_1'][:],
                                             start=True, stop=True)
                            if "KNOCOPY" not in os.environ:
                                srcap = vp[:].rearrange(
                                    "p (kk it) -> p kk it", kk=32)\
                                    .rearrange("p kk it -> p it kk")
                                dst = vsb[1][:].rearrange(
                                    "p (it kk) -> p it kk", kk=32)
                                nc.scalar.copy(dst, srcap)
                    # gather
                    G = ps3.tile([128, GROW], BF16, tag="G")
                    if PH >= 4:
                        nc.gpsimd.indirect_dma_start(
                            out=G[:], out_offset=None, in_=gtab.ap()[:, :],
                            in_offset=IndirectOffsetOnAxis(
                                ap=nbr_all[:, ti:ti + 1], axis=0))
                    else:
                        nc.gpsimd.memset(G[:], 0.0)
                    # products
                    PT = [ps3.tile([128, 1408], BF16, tag=f"PT{t}", name=f"PT{t}")
                          for t in range(5)]
                    for t in range(5) if KS >= 4 else []:
                        v = vsb3[:].rearrange(
                            "p (i t k) -> p i t k", i=5, k=KW)\
                            [:, :, t, :].unsqueeze(2)\
                            .broadcast_to([128, 5, 5, KW])
                        g3 = G[:, GOFF[3]:GOFF[3] + 800].rearrange(
                            "p (t j k) -> p t j k", t=5, k=KW)\
                            [:, t, :, :].unsqueeze(1)\
                            .broadcast_to([128, 5, 5, KW])
                        o = PT[t][:, 0:800].rearrange(
                            "p (i j k) -> p i j k", i=5, k=KW)
                        nc.vector.tensor_tensor(out=o, in0=v, in1=g3,
                                                op=mybir.AluOpType.mult)
                    for l in (2, 1) if KS >= 4 else []:
                        for t in range(3):
                            v = vsb[l][:].rearrange(
                                "p (i t k) -> p i t k", i=3, k=KW)\
                                [:, :, t, :].unsqueeze(2)\
                                .broadcast_to([128, 3, 3, KW])
                            gl = G[:, GOFF[l]:GOFF[l] + 288].rearrange(
                                "p (t j k) -> p t j k", t=3, k=KW)\
                                [:, t, :, :].unsqueeze(1)\
                                .broadcast_to([128, 3, 3, KW])
                            o = PT[t][:, GOFF[l]:GOFF[l] + 288]\
                                .rearrange("p (i j k) -> p i j k", i=3, k=KW)
                            nc.vector.tensor_tensor(out=o, in0=v, in1=gl,
                                                    op=mybir.AluOpType.mult)
                    if KS >= 4:
                     nc.vector.tensor_tensor(
                        out=PT[0][:, 1376:1408], in0=cvec[:, 928:960],
                        in1=G[:, GOFF[0]:GOFF[0] + KW],
                        op=mybir.AluOpType.mult)
                    # segment matmuls
                    if KS < 5:
                        continue
                    lhs_ind = ind_all[:, ti, :]
                    for t in range(5):
                        for h in range(2):
                            nc.tensor.matmul(
                                np3[h][:], lhsT=lhs_ind,
                                rhs=PT[t][:, h * 400:h * 400 + 400],
                                start=(first and t == 0),
                                stop=(last and t == 4))
                    for t in range(3):
                        nc.tensor.matmul(np2[:], lhsT=lhs_ind,
                                         rhs=PT[t][:, 800:1088],
                                         start=(first and t == 0),
                                         stop=(last and t == 2))
                    nc.tensor.matmul(np10[:], lhsT=lhs_ind,
                                     rhs=PT[0][:, 1088:1408],
                                     start=first, stop=False)
                    for t in (1, 2):
                        nc.tensor.matmul(np10[:, :288], lhsT=lhs_ind,
                                         rhs=PT[t][:, 1088:1376],
                                         start=False, stop=(last and t == 2))
                # ---- chunk epilogue ----
                if int(os.environ.get("KSTAGE", "9")) < 5:
                    continue
                pooled = pout.tile([128, PTOT], BF16, tag="pooled")
                nc.scalar.copy(pooled[:, 0:400], np3[0][:])
                nc.scalar.copy(pooled[:, 400:800], np3[1][:])
                nc.scalar.copy(pooled[:, 800:1088], np2[:])
                nc.scalar.copy(pooled[:, 1088:1408], np10[:])
                for p in range(NPIECE):
                    tp = pp3.tile([128, 128], BF16, tag="tp", bufs=2)
                    nc.tensor.transpose(tp[:], pooled[:, p * 128:p * 128 + 128],
                                        ident[:])
                    nc.scalar.copy(
                        pieces[:, p, s_i * 128:s_i * 128 + 128], tp[:])
            # ---- output stage (all chunks) ----
            if int(os.environ.get("KSTAGE", "9")) >= 5:
                AC = NCH * 128
                for l in range(4):
                    ncol = (2 * l + 1) * KMAX[l]
                    fo = pout.tile([128, NCH, 960], F32, tag="fo")
                    nc.sync.dma_start(
                        fo[:, :, :ncol],
                        din[f'featown_{l}'].ap().rearrange(
                            "(s q) c -> q s c", q=128))
                    for c0 in range(0, ncol, 128):
                        cw = min(128, ncol - c0)
                        ops_t = pp3.tile([128, 400], F32, tag="vv", bufs=2,
                                         name="ops")
                        ops = ops_t[:, 0:AC]
                        for p in range(NPIECE):
                            nc.tensor.matmul(
                                ops[:cw, :],
                                lhsT=sb[f'WU_{l}'][:, p, c0:c0 + cw],
                                rhs=pieces[:, p, :],
                                start=(p == 0), stop=(p == NPIECE - 1))
                        osb = pout.tile([128, 400], BF16, tag="osb")
                        nc.scalar.copy(osb[:cw, :AC], ops[:cw, :])
                        for s_i in range(NCH):
                            tp2 = pp3.tile([128, 128], BF16, tag="tp", bufs=2)
                            nc.tensor.transpose(
                                tp2[:, :cw],
                                osb[:cw, s_i * 128:s_i * 128 + 128],
                                ident[:cw, :cw])
                            ofin = pout.tile([128, 128], F32, tag="ofin",
                                             bufs=2)
                            nc.vector.tensor_add(out=ofin[:, :cw],
                                                 in0=tp2[:, :cw],
                                                 in1=fo[:, s_i, c0:c0 + cw])
                            nc.sync.dma_start(
                                douts[l].ap()[s_i * 128:s_i * 128 + 128,
                                              c0:c0 + cw], ofin[:, :cw])
        ctx.close()
    nc.compile()
    return nc


def kernel(**inputs):
    per_core, rep, meta = _host_prep(inputs)
    nc = build_program(meta)
    in_maps = []
    for c in range(NC_):
        m = dict(per_core[c])
        m.update(rep)
        in_maps.append(m)
    res = run_bass_kernel_spmd(nc, in_maps, list(range(NC_)))
    outs = []
    abnd = meta['abnd']
    for l in range(4):
        full = np.zeros((N_ATOMS, 2 * l + 1, KMAX[l]), np.float32)
        for c in range(NC_):
            a0, a1 = abnd[c], abnd[c + 1]
            full[a0:a1] = res.results[c][f'out_{l}'][:a1 - a0].reshape(
                a1 - a0, 2 * l + 1, KMAX[l])
        outs.append(full)
    return tuple(outs)
